# revision 1
# baseline (speedup 1.0000x reference)
"""GAT+LSTM fused kernel for 8 trn2 NeuronCores.

- Output depends only on batch row T-1=11 of the reference LSTM (ys[:, -1]),
  so only GAT outputs for nodes [110000, 120000) ("live" nodes) are needed.
- Edges sharded by src-range across 8 cores (only edges with a live dst);
  self-loops are injected as ordinary edges (their edge_attr slot is filled
  on-device with mean(edge_attr) after a tiny AllReduce).
- Per-edge rows fetched with dma_gather from per-core blocked node tables;
  segment softmax without max-subtraction (numerically safe here).
- Aggregation: host groups each core's edges by destination bucket
  (dst >> 7), exactly 2 chunks of 128 per bucket (canonical schedule, same
  for every core); on-chip each chunk builds a one-hot (dst == iota) matrix
  and a PE matmul accumulates messages into the bucket's PSUM tile.
- Partial accumulators combined with ReduceScatter; normalize + ReLU +
  transpose; AllGather; LSTM input projection.
- LSTM solved by fixed-point iteration (ITERS full-sequence passes) with
  tensor_tensor_scan for the cell recurrence; converges to the exact serial
  recurrence to f32 noise in ~12 passes.
"""
import os
import numpy as np

import concourse.bass as bass
import concourse.bacc as bacc
import concourse.tile as tile
from concourse import mybir
from concourse.bass_utils import run_bass_kernel_spmd
from concourse.masks import make_identity
from contextlib import ExitStack

dt = mybir.dt
F32 = dt.float32
I16 = dt.int16
AF = mybir.ActivationFunctionType
ALU = mybir.AluOpType

T, N, F_IN = 12, 10000, 64
HEADS, C, HID = 4, 32, 32
E, TN = 1_000_000, 120_000
NCORES = 8
NSH = TN // NCORES              # 15000 nodes per x-shard
NBLK = 118                      # main node-table blocks (118*128 = 15104)
NSHP = NBLK * 128
D0 = (T - 1) * N
DN = N
DBLK = 80                       # live-node buckets (80*128 = 10240)
DNP = DBLK * 128
DSL = DNP // NCORES             # 1280 live nodes owned per core
SBLK = 10                       # appended self-node blocks (10*128 = 1280)
NROWS = NSHP + SBLK * 128       # 16384 rows in h/asrc tables
BSLOT = 256                     # canonical slots per bucket (2 chunks)
NCH = DBLK * 2                  # 160 chunks
CAP = NCH * 128                 # 20480
NPASS = 2                       # edge phase in 2 passes of 80 chunks
NT = DNP
SC = 2048
NSC = NT // SC
ITERS = 8
LEAK = 0.2

_CACHE = {}


def _wrap16(idx, cap):
    out = np.zeros((16, cap // 16), np.int16)
    j = np.arange(len(idx))
    out[j % 16, j // 16] = np.asarray(idx).astype(np.int16)
    return np.tile(out, (8, 1))


def _chunkify(vals, cap, fill):
    out = np.full(cap, fill, np.float32)
    out[:len(vals)] = vals
    return np.ascontiguousarray(out.reshape(cap // 128, 128).T)


def _row_main(n):
    return (n % 128) * NBLK + n // 128


def _row_self(c):
    return NSHP + (c % 128) * SBLK + c // 128


def _row_ad(n):
    return (n % 128) * DBLK + n // 128


def _prep_host(inputs):
    x = np.ascontiguousarray(np.asarray(inputs["x_seq"], np.float32).reshape(TN, F_IN))
    ei = np.asarray(inputs["edge_index"])
    ea = np.asarray(inputs["edge_attr"], np.float32)[:, 0]
    W_gat = np.asarray(inputs["W_gat"], np.float32)
    att_src = np.asarray(inputs["att_src"], np.float32)
    att_dst = np.asarray(inputs["att_dst"], np.float32)
    att_edge = np.asarray(inputs["att_edge"], np.float32)
    W_edge = np.asarray(inputs["W_edge"], np.float32)
    gat_bias = np.asarray(inputs["gat_bias"], np.float32)
    W_ih = np.asarray(inputs["W_ih"], np.float32)
    W_hh = np.asarray(inputs["W_hh"], np.float32)
    b = np.asarray(inputs["b_ih"], np.float32) + np.asarray(inputs["b_hh"], np.float32)
    W_fc = np.asarray(inputs["W_fc"], np.float32)
    b_fc = np.asarray(inputs["b_fc"], np.float32)

    A_src = np.zeros((HEADS * C, HEADS), np.float32)
    A_dst = np.zeros((HEADS * C, HEADS), np.float32)
    for h in range(HEADS):
        A_src[h * C:(h + 1) * C, h] = att_src[h]
        A_dst[h * C:(h + 1) * C, h] = att_dst[h]
    Wa8 = np.concatenate([W_gat @ A_src, W_gat @ A_dst], axis=1)
    kap = np.array([np.dot(W_edge[0, h * C:(h + 1) * C], att_edge[h])
                    for h in range(HEADS)], np.float32)
    kap_rep = np.broadcast_to(kap, (128, HEADS)).copy()
    gbias_rep = np.broadcast_to(gat_bias, (128, HEADS * C)).copy()
    iota128 = np.broadcast_to(np.arange(128, dtype=np.float32), (128, 128)).copy()
    perm = np.concatenate([np.arange(32, 64), np.arange(0, 32),
                           np.arange(96, 128), np.arange(64, 96)])
    WihT = np.ascontiguousarray(W_ih[perm].T)
    WhhT = np.ascontiguousarray(W_hh[perm].T)
    br = np.ascontiguousarray(b[perm].reshape(128, 1))

    src, dst = ei[0].astype(np.int64), ei[1].astype(np.int64)
    live = (dst >= D0) & (dst < D0 + DN)
    core_of = src // NSH
    xTD = np.zeros((F_IN, DNP), np.float32)
    xTD[:, :DN] = x[D0:D0 + DN].T
    ZN, ZD = NSH, DN + 100      # zero-content pad nodes (shard / live)
    in_maps = []
    ea_all = ea.reshape(NCORES, E // NCORES)
    for k in range(NCORES):
        m = live & (core_of == k)
        sL = src[m] - k * NSH
        dL = dst[m] - D0
        eav = ea[m]
        own = np.arange(DNP).reshape(DBLK, 8, 16)[:, k, :].ravel()
        own_cols = 16 * (own // 128) + (own % 128 - 16 * k)
        o2 = np.argsort(own_cols)
        own = own[o2]                       # xTDS column c <-> live node own[c]
        xTDS = np.ascontiguousarray(xTD[:, own])
        bflat = dL >> 7
        hs_idx = np.full(CAP, _row_main(ZN), np.int64)
        ad_idx = np.full(CAP, _row_ad(ZD), np.int64)
        eac = np.zeros(CAP, np.float32)
        dstf = np.full(CAP, -1.0, np.float32)
        selfm = np.zeros(CAP, np.float32)
        for bkt in range(DBLK):
            sel = np.nonzero(bflat == bkt)[0]
            sn = own[(own >= 128 * bkt) & (own < 128 * (bkt + 1))]
            cn = 16 * (sn // 128) + (sn % 128 - 16 * k)
            nb = len(sel) + len(sn)
            assert nb <= BSLOT, f"core {k} bucket {bkt}: {nb} > {BSLOT}"
            o = bkt * BSLOT
            ne = len(sel)
            hs_idx[o:o + ne] = _row_main(sL[sel])
            ad_idx[o:o + ne] = _row_ad(dL[sel])
            dstf[o:o + ne] = dL[sel] - 128 * bkt
            eac[o:o + ne] = eav[sel]
            hs_idx[o + ne:o + nb] = _row_self(cn)
            ad_idx[o + ne:o + nb] = _row_ad(sn)
            dstf[o + ne:o + nb] = sn - 128 * bkt
            selfm[o + ne:o + nb] = 1.0
        xT = np.zeros((F_IN, NSHP), np.float32)
        xT[:, :NSH] = x[k * NSH:(k + 1) * NSH].T
        eaF = np.zeros((128, 980), np.float32)
        ch = ea_all[k]
        j2 = np.arange(len(ch))
        eaF[j2 % 128, j2 // 128] = ch
        in_maps.append({
            "xT": xT, "xTD": xTD, "xTDS": xTDS,
            "eaC": _chunkify(eac, CAP, 0.0),
            "dstF": _chunkify(dstf, CAP, -1.0),
            "selfM": _chunkify(selfm, CAP, 0.0),
            "eaF": eaF,
            "hsI": np.ascontiguousarray(
                hs_idx.reshape(NCH, 128).T).astype(np.int32),
            "adI": np.ascontiguousarray(
                ad_idx.reshape(NCH, 128).T).astype(np.int32),
            "Wgat": W_gat, "Wa8": Wa8, "kap": kap_rep, "gbias": gbias_rep,
            "iota": iota128,
            "Wih": WihT, "Whh": WhhT, "br": br,
            "Wfc": np.ascontiguousarray(W_fc.reshape(HID, 1)),
            "bfc": np.ascontiguousarray(b_fc.reshape(1, 1)),
        })
    return in_maps


def _build_nc(debug=False):
    STAGE = int(os.environ.get("KSTAGE", "99"))
    nc = bacc.Bacc("TRN2", target_bir_lowering=False, debug=False,
                   num_devices=NCORES)
    g = lambda n, s, d=F32: nc.dram_tensor(n, s, d, kind="ExternalInput").ap()
    xT = g("xT", [F_IN, NSHP]); xTD = g("xTD", [F_IN, DNP])
    xTDS = g("xTDS", [F_IN, DSL])
    eaC = g("eaC", [128, NCH]); dstF = g("dstF", [128, NCH])
    selfM = g("selfM", [128, NCH]); eaF = g("eaF", [128, 980])
    hsI = g("hsI", [128, NCH], dt.int32); adI = g("adI", [128, NCH], dt.int32)
    Wgat = g("Wgat", [F_IN, 128]); Wa8 = g("Wa8", [F_IN, 8])
    kap = g("kap", [128, HEADS]); gbias = g("gbias", [128, 128])
    iota = g("iota", [128, 128])
    Wih = g("Wih", [128, 128]); Whh = g("Whh", [HID, 128])
    br = g("br", [128, 1]); Wfc = g("Wfc", [HID, 1]); bfc = g("bfc", [1, 1])
    out = nc.dram_tensor("out", [1, NT], F32, kind="ExternalOutput").ap()
    if debug:
        dbg_gat = nc.dram_tensor("dbg_gat", [128, DSL], F32, kind="ExternalOutput").ap()
        dbg_gx = nc.dram_tensor("dbg_gx", [128, NT], F32, kind="ExternalOutput").ap()
        dbg_h = nc.dram_tensor("dbg_h", [HID, NT], F32, kind="ExternalOutput").ap()

    h_tbl = nc.dram_tensor("h_tbl", [NROWS, 192], F32).ap()
    ad_tbl = nc.dram_tensor("ad_tbl", [DNP, 64], F32).ap()
    acc_tbl = nc.dram_tensor("acc_tbl", [128, DBLK * 132], F32).ap()
    ea_in = nc.dram_tensor("ea_in", [128, 1], F32).ap()
    ea_out = nc.dram_tensor("ea_out", [128, 1], F32, addr_space="Shared").ap()
    rs_out = nc.dram_tensor("rs_out", [16, DBLK * 132], F32).ap()
    gat_blob = nc.dram_tensor("gat_blob", [128, DSL], F32).ap()
    gat_full = nc.dram_tensor("gat_full", [NCORES, 128, DSL], F32,
                              addr_space="Shared").ap()
    RG = [list(range(NCORES))]

    with tile.TileContext(nc) as tc, ExitStack() as top:
        const = top.enter_context(tc.tile_pool(name="const", bufs=1))
        ident = const.tile([128, 128], F32)
        make_identity(nc, ident[:])
        wgat_t = const.tile([F_IN, 128], F32); nc.sync.dma_start(wgat_t[:], Wgat[:])
        wa8_t = const.tile([F_IN, 8], F32); nc.sync.dma_start(wa8_t[:], Wa8[:])
        kap_t = const.tile([128, HEADS], F32); nc.sync.dma_start(kap_t[:], kap[:])
        gb_t = const.tile([128, 128], F32); nc.sync.dma_start(gb_t[:], gbias[:])
        iota_t = const.tile([128, 128], F32); nc.sync.dma_start(iota_t[:], iota[:])
        whh_t = const.tile([HID, 128], F32); nc.sync.dma_start(whh_t[:], Whh[:])
        wih_t = const.tile([128, 128], F32); nc.sync.dma_start(wih_t[:], Wih[:])
        br_t = const.tile([128, 1], F32); nc.sync.dma_start(br_t[:], br[:])
        wfc_t = const.tile([HID, 1], F32); nc.sync.dma_start(wfc_t[:], Wfc[:])
        bfc_t = const.tile([1, 1], F32); nc.sync.dma_start(bfc_t[:], bfc[:])
        meanr = const.tile([128, 1], F32)

        # ---------- A1: mean(edge_attr) via AllReduce + PE broadcast -------
        with ExitStack() as ph:
            sbm = ph.enter_context(tc.tile_pool(name="sbm", bufs=1))
            psm = ph.enter_context(tc.tile_pool(name="psm", bufs=1, space="PSUM"))
            eaf_t = sbm.tile([128, 980], F32)
            nc.sync.dma_start(eaf_t[:], eaF[:])
            eap = sbm.tile([128, 1], F32)
            nc.vector.tensor_reduce(eap[:], eaf_t[:], mybir.AxisListType.X, ALU.add)
            nc.sync.dma_start(ea_in[:], eap[:])
            nc.gpsimd.collective_compute("AllReduce", ALU.add, replica_groups=RG,
                                         ins=[ea_in[:]], outs=[ea_out[:]])
            eao_t = sbm.tile([128, 1], F32)
            nc.sync.dma_start(eao_t[:], ea_out[:])
            onc = sbm.tile([128, 1], F32)
            nc.gpsimd.memset(onc[:], 1.0)
            ps1 = psm.tile([1, 1], F32, space="PSUM", tag="ps1")
            nc.tensor.matmul(ps1[:], lhsT=eao_t[:], rhs=onc[:], start=True, stop=True)
            eas = sbm.tile([1, 1], F32)
            nc.scalar.mul(eas[:], ps1[:], 1.0 / E)
            onr = sbm.tile([1, 128], F32)
            nc.gpsimd.memset(onr[:], 1.0)
            ps2 = psm.tile([128, 1], F32, space="PSUM", tag="ps2")
            nc.tensor.matmul(ps2[:], lhsT=onr[:], rhs=eas[:], start=True, stop=True)
            nc.vector.tensor_copy(meanr[:], ps2[:])

        if STAGE >= 1:
            # ---------- A2/A3: blocked node tables ----------
            with ExitStack() as ph:
                sba = ph.enter_context(tc.tile_pool(name="sba", bufs=1))
                psa = ph.enter_context(tc.tile_pool(name="psa", bufs=4, space="PSUM"))
                psp = ph.enter_context(tc.tile_pool(name="psp", bufs=1, space="PSUM"))
                xt_t = sba.tile([F_IN, NSHP], F32)
                nc.sync.dma_start(xt_t[:], xT[:])
                HS = sba.tile([128, NBLK * 192], F32)
                HSv = HS[:].rearrange("p (j w) -> p j w", w=192)
                pack_a = psp.tile([128, NBLK * 4], F32, space="PSUM", tag="pka")
                for j in range(NBLK):
                    ph_ = psa.tile([128, 128], F32, space="PSUM", tag="ph")
                    nc.tensor.matmul(ph_[:], lhsT=xt_t[:, j * 128:(j + 1) * 128],
                                     rhs=wgat_t[:], start=True, stop=True)
                    nc.vector.tensor_copy(HSv[:, j, 0:128], ph_[:])
                    nc.tensor.matmul(pack_a[:, j * 4:(j + 1) * 4],
                                     lhsT=xt_t[:, j * 128:(j + 1) * 128],
                                     rhs=wa8_t[:, 0:4], start=True, stop=True)
                nc.vector.tensor_copy(HSv[:, :, 128:132],
                                      pack_a[:].rearrange("p (j w) -> p j w", w=4))
                nc.sync.dma_start(h_tbl[0:NSHP, :].rearrange("(p j) w -> p (j w)", p=128),
                                  HS[:])
            with ExitStack() as ph:
                sbb = ph.enter_context(tc.tile_pool(name="sbb", bufs=1))
                psb = ph.enter_context(tc.tile_pool(name="psb", bufs=4, space="PSUM"))
                psq = ph.enter_context(tc.tile_pool(name="psq", bufs=1, space="PSUM"))
                xtd_t = sbb.tile([F_IN, DNP], F32)
                nc.sync.dma_start(xtd_t[:], xTD[:])
                SAD = sbb.tile([128, DBLK * 64], F32)
                nc.gpsimd.memset(SAD[:], 0.0)
                pack_d = psq.tile([128, DBLK * 4], F32, space="PSUM", tag="pkd")
                for j in range(DBLK):
                    nc.tensor.matmul(pack_d[:, j * 4:(j + 1) * 4],
                                     lhsT=xtd_t[:, j * 128:(j + 1) * 128],
                                     rhs=wa8_t[:, 4:8], start=True, stop=True)
                SADv = SAD[:].rearrange("p (j w) -> p j w", w=64)
                nc.vector.tensor_copy(SADv[:, :, 0:4],
                                      pack_d[:].rearrange("p (j w) -> p j w", w=4))
                nc.sync.dma_start(ad_tbl[:].rearrange("(p j) w -> p (j w)", p=128),
                                  SAD[:])
                # appended self-node rows (from xTDS)
                xts_t = sbb.tile([F_IN, DSL], F32)
                nc.sync.dma_start(xts_t[:], xTDS[:])
                HS2 = sbb.tile([128, SBLK * 192], F32)
                HS2v = HS2[:].rearrange("p (j w) -> p j w", w=192)
                pack_s = psq.tile([128, SBLK * 4], F32, space="PSUM", tag="pks")
                for j in range(SBLK):
                    ph2 = psb.tile([128, 128], F32, space="PSUM", tag="ph2")
                    nc.tensor.matmul(ph2[:], lhsT=xts_t[:, j * 128:(j + 1) * 128],
                                     rhs=wgat_t[:], start=True, stop=True)
                    nc.vector.tensor_copy(HS2v[:, j, 0:128], ph2[:])
                    nc.tensor.matmul(pack_s[:, j * 4:(j + 1) * 4],
                                     lhsT=xts_t[:, j * 128:(j + 1) * 128],
                                     rhs=wa8_t[:, 0:4], start=True, stop=True)
                nc.vector.tensor_copy(HS2v[:, :, 128:132],
                                      pack_s[:].rearrange("p (j w) -> p j w", w=4))
                nc.sync.dma_start(
                    h_tbl[NSHP:NROWS, :].rearrange("(p j) w -> p (j w)", p=128), HS2[:])

        if STAGE >= 2:
            # ---------- A5: edge phase, 2 passes of 80 chunks ----------
            PC = NCH // NPASS               # 80 chunks per pass
            PCAP = PC * 128                 # 10240 idx per pass
            with ExitStack() as ph:
                sbe = ph.enter_context(tc.tile_pool(name="sbe", bufs=1))
                ACCT = sbe.tile([128, DBLK * 132], F32)
                hsI_t = sbe.tile([128, NCH], dt.int32)
                nc.sync.dma_start(hsI_t[:], hsI[:])
                adI_t = sbe.tile([128, NCH], dt.int32)
                nc.sync.dma_start(adI_t[:], adI[:])
                ea_t = sbe.tile([128, NCH], F32)
                nc.sync.dma_start(ea_t[:], eaC[:])
                df_t = sbe.tile([128, NCH], F32)
                nc.sync.dma_start(df_t[:], dstF[:])
                sm_t = sbe.tile([128, NCH], F32)
                nc.sync.dma_start(sm_t[:], selfM[:])
                for p in range(NPASS):
                    with ExitStack() as pp:
                        sbp = pp.enter_context(tc.tile_pool(name=f"sbp{p}", bufs=1))
                        sbo = pp.enter_context(tc.tile_pool(name=f"sbo{p}", bufs=4))
                        pse = pp.enter_context(tc.tile_pool(name=f"pse{p}", bufs=4,
                                                            space="PSUM"))
                        c0 = p * PC
                        NH = sbp.tile([128, PC * 192], F32, tag="NH")
                        AD = sbp.tile([128, PC * 64], F32, tag="AD")
                        SCT = sbp.tile([128, PC * 132], F32, tag="SCT")
                        S4 = sbp.tile([128, PC * 4], F32, tag="S4")
                        NHv = NH[:].rearrange("p (e w) -> p e w", w=192)
                        ADv = AD[:].rearrange("p (e w) -> p e w", w=64)
                        SCv = SCT[:].rearrange("p (e w) -> p e w", w=132)
                        S4v = S4[:].rearrange("p (e w) -> p e w", w=4)
                        for cc in range(PC):
                            nc.gpsimd.indirect_dma_start(
                                out=NHv[:, cc, :], out_offset=None, in_=h_tbl[:],
                                in_offset=bass.IndirectOffsetOnAxis(
                                    ap=hsI_t[:, c0 + cc:c0 + cc + 1], axis=0))
                            nc.gpsimd.indirect_dma_start(
                                out=ADv[:, cc, :], out_offset=None, in_=ad_tbl[:],
                                in_offset=bass.IndirectOffsetOnAxis(
                                    ap=adI_t[:, c0 + cc:c0 + cc + 1], axis=0))
                        # ea' = ea + selfM*mean ; q = a_src+a_dst+ea'*kap
                        EAm = sbp.tile([128, PC], F32, tag="EAm")
                        nc.vector.scalar_tensor_tensor(
                            out=EAm[:], in0=sm_t[:, c0:c0 + PC], scalar=meanr[:],
                            op0=ALU.mult, op1=ALU.add, in1=ea_t[:, c0:c0 + PC])
                        Q = sbp.tile([128, PC * 4], F32, tag="Q")
                        Qv = Q[:].rearrange("p (e w) -> p e w", w=4)
                        nc.vector.tensor_tensor(out=Qv, in0=NHv[:, :, 128:132],
                                                in1=ADv[:, :, 0:4], op=ALU.add)
                        T2 = sbp.tile([128, PC * 4], F32, tag="T2")
                        T2v = T2[:].rearrange("p (e w) -> p e w", w=4)
                        ea3 = EAm[:].rearrange("p (e w) -> p e w", w=1) \
                            .to_broadcast([128, PC, 4])
                        kap3 = kap_t[:].rearrange("p (o w) -> p o w", o=1) \
                            .to_broadcast([128, PC, 4])
                        nc.vector.tensor_tensor(out=T2v, in0=ea3, in1=kap3, op=ALU.mult)
                        nc.vector.tensor_tensor(out=Qv, in0=Qv, in1=T2v, op=ALU.add)
                        nc.vector.tensor_scalar_mul(T2v, Qv, LEAK)
                        nc.vector.tensor_tensor(out=Qv, in0=Qv, in1=T2v, op=ALU.max)
                        # exp(q) = sigmoid(q)/sigmoid(-q); Exp's ACT table
                        # is not resident (measured ~100x slowdown)
                        SG1 = sbp.tile([128, PC * 4], F32, tag="SG1")
                        nc.scalar.activation(SG1[:], Q[:], AF.Sigmoid)
                        nc.scalar.activation(S4[:], Q[:], AF.Sigmoid, scale=-1.0)
                        nc.vector.reciprocal(S4[:], S4[:])
                        nc.vector.tensor_tensor(out=S4[:], in0=SG1[:], in1=S4[:],
                                                op=ALU.mult)
                        nc.vector.tensor_copy(SCv[:, :, 128:132], S4v)
                        nh4 = NHv[:, :, 0:128].rearrange("p e (h c) -> p e h c", h=HEADS)
                        sc4 = S4v.rearrange("p e (h c) -> p e h c", c=1) \
                            .to_broadcast([128, PC, HEADS, C])
                        out4 = SCv[:, :, 0:128].rearrange("p e (h c) -> p e h c", h=HEADS)
                        nc.vector.tensor_tensor(out=out4, in0=nh4, in1=sc4, op=ALU.mult)
                        # one-hot binning into per-bucket PSUM accumulators
                        for c80 in range(PC):
                            cg = c0 + c80
                            bkt = cg // 2
                            first = (cg % 2 == 0)
                            last = (cg % 2 == 1)
                            oh = sbo.tile([128, 128], F32, tag="oh")
                            nc.vector.tensor_tensor(
                                out=oh[:],
                                in0=df_t[:, cg:cg + 1].to_broadcast([128, 128]),
                                in1=iota_t[:], op=ALU.is_equal)
                            if first:
                                pacc = pse.tile([128, 132], F32, space="PSUM", tag="pacc")
                            nc.tensor.matmul(pacc[:], lhsT=oh[:], rhs=SCv[:, c80, :],
                                             start=first, stop=last)
                            if last:
                                nc.vector.tensor_copy(
                                    ACCT[:, bkt * 132:(bkt + 1) * 132], pacc[:])
                nc.sync.dma_start(acc_tbl[:], ACCT[:])

        if STAGE >= 3:
            # ---------- A6: ReduceScatter ----------
            nc.gpsimd.collective_compute("ReduceScatter", ALU.add, replica_groups=RG,
                                         ins=[acc_tbl[:]], outs=[rs_out[:]])

        if STAGE >= 3:
            # ---------- A7: normalize + ReLU + transpose my slice ----------
            with ExitStack() as ph:
                sbn = ph.enter_context(tc.tile_pool(name="sbn", bufs=1))
                sbw = ph.enter_context(tc.tile_pool(name="sbw", bufs=4))
                psn = ph.enter_context(tc.tile_pool(name="psn", bufs=4, space="PSUM"))
                RSS = sbn.tile([16, DBLK * 132], F32)
                nc.sync.dma_start(RSS[:], rs_out[:])
                RSv = RSS[:].rearrange("p (j w) -> p j w", w=132)
                nc.vector.tensor_scalar_add(RSv[:, :, 128:132], RSv[:, :, 128:132], 1e-16)
                RC = sbn.tile([16, DBLK * 4], F32)
                RCv = RC[:].rearrange("p (j w) -> p j w", w=4)
                nc.vector.reciprocal(RCv, RSv[:, :, 128:132])
                r4 = RCv.rearrange("p j (h c) -> p j h c", c=1) \
                    .to_broadcast([16, DBLK, HEADS, C])
                m4 = RSv[:, :, 0:128].rearrange("p j (h c) -> p j h c", h=HEADS)
                nc.vector.tensor_tensor(out=m4, in0=m4, in1=r4, op=ALU.mult)
                gbb = gb_t[0:16, :].rearrange("p (o w) -> p o w", o=1) \
                    .to_broadcast([16, DBLK, 128])
                nc.vector.tensor_tensor(out=RSv[:, :, 0:128], in0=RSv[:, :, 0:128],
                                        in1=gbb, op=ALU.add)
                nc.vector.tensor_scalar_max(RSv[:, :, 0:128], RSv[:, :, 0:128], 0.0)
                GB = sbn.tile([128, DSL], F32)
                for j in range(DBLK):
                    ptr = psn.tile([128, 16], F32, space="PSUM", tag="ptr")
                    nc.tensor.transpose(out=ptr[:], in_=RSv[:, j, 0:128],
                                        identity=ident[0:16, 0:16])
                    nc.vector.tensor_copy(GB[:, j * 16:(j + 1) * 16], ptr[:])
                nc.sync.dma_start(gat_blob[:], GB[:])
        if debug:
            nc.sync.dma_start(dbg_gat[:], gat_blob[:])

        if STAGE >= 4:
            # ---------- A8: AllGather ----------
            nc.gpsimd.collective_compute("AllGather", ALU.bypass, replica_groups=RG,
                                         ins=[gat_blob[:]], outs=[gat_full[:]])

            # ---------- A9: gx = W_ih @ gat + b ----------
        persist = top.enter_context(tc.tile_pool(name="persist", bufs=1))
        gx = persist.tile([128, NT], F32)
        H = persist.tile([HID, NT + 32], F32)
        nc.gpsimd.memset(H[:], 0.0)
        if STAGE < 5:
            nc.gpsimd.memset(gx[:], 0.0)
        if STAGE >= 5:
            with ExitStack() as ph:
                sbg = ph.enter_context(tc.tile_pool(name="sbg", bufs=1))
                psg = ph.enter_context(tc.tile_pool(name="psg", bufs=4, space="PSUM"))
                GT = sbg.tile([128, NT], F32)
                GTv = GT[:].rearrange("p (j b r) -> p j b r", b=NCORES, r=16)
                for kk in range(NCORES):
                    nc.sync.dma_start(
                        GTv[:, :, kk, :],
                        gat_full[kk].rearrange("p (j r) -> p j r", r=16))
                for c in range(NT // 512):
                    pg = psg.tile([128, 512], F32, space="PSUM", tag="pg")
                    nc.tensor.matmul(pg[:], lhsT=wih_t[:], rhs=GT[:, c * 512:(c + 1) * 512],
                                     start=True, stop=True)
                    nc.vector.tensor_scalar_add(gx[:, c * 512:(c + 1) * 512], pg[:], br_t[:])
        if debug:
            nc.sync.dma_start(dbg_gx[:], gx[:])

        if STAGE >= 6:
            # ---------- A10: LSTM fixed point ----------
            with ExitStack() as ph:
                sbl = ph.enter_context(tc.tile_pool(name="sbl", bufs=2))
                sbl3 = ph.enter_context(tc.tile_pool(name="sbl3", bufs=3))
                psl = ph.enter_context(tc.tile_pool(name="psl", bufs=2, space="PSUM"))
                for it in range(ITERS):
                    Cprev = None
                    for s in range(NSC):
                        lo, hi = s * SC, (s + 1) * SC
                        if it == 0:
                            Gp = gx[:, lo:hi]
                        else:
                            pG = psl.tile([128, SC], F32, space="PSUM", tag="pG")
                            for q in range(SC // 512):
                                nc.tensor.matmul(pG[:, q * 512:(q + 1) * 512], lhsT=whh_t[:],
                                                 rhs=H[:, lo + q * 512:lo + (q + 1) * 512],
                                                 start=True, stop=True)
                            Gs = sbl.tile([128, SC], F32, tag="Gs")
                            nc.vector.tensor_tensor(out=Gs[:], in0=pG[:], in1=gx[:, lo:hi],
                                                    op=ALU.add)
                            Gp = Gs[:]
                        S_ = sbl.tile([96, SC], F32, tag="S")
                        nc.scalar.activation(S_[:], Gp[0:96, :], AF.Sigmoid)
                        Tg = sbl.tile([64, SC], F32, tag="Tg")
                        nc.scalar.activation(Tg[32:64, :], Gp[96:128, :], AF.Tanh)
                        Zt = sbl3.tile([HID, SC], F32, tag="Zt")
                        nc.vector.tensor_tensor(out=Zt[:], in0=S_[32:64, :],
                                                in1=Tg[32:64, :], op=ALU.mult)
                        Ct = sbl3.tile([HID, SC], F32, tag="Ct")
                        nc.vector.tensor_tensor_scan(
                            out=Ct[:], data0=S_[0:32, :], data1=Zt[:],
                            initial=(0.0 if Cprev is None else Cprev[:, SC - 1:SC]),
                            op0=ALU.mult, op1=ALU.add)
                        TC = sbl.tile([96, SC], F32, tag="TC")
                        nc.scalar.activation(TC[64:96, :], Ct[:], AF.Tanh)
                        nc.vector.tensor_tensor(out=H[:, lo + 1:hi + 1], in0=S_[64:96, :],
                                                in1=TC[64:96, :], op=ALU.mult)
                        Cprev = Ct
        if debug:
            nc.sync.dma_start(dbg_h[:], H[:, 1:NT + 1])

        if STAGE >= 7:
            # ---------- FC ----------
            with ExitStack() as ph:
                sbf = ph.enter_context(tc.tile_pool(name="sbf", bufs=1))
                psf = ph.enter_context(tc.tile_pool(name="psf", bufs=4, space="PSUM"))
                OF = sbf.tile([1, NT], F32)
                for c in range(NT // 512):
                    pf = psf.tile([1, 512], F32, space="PSUM", tag="pf")
                    nc.tensor.matmul(pf[:], lhsT=wfc_t[:],
                                     rhs=H[:, 1 + c * 512:1 + (c + 1) * 512],
                                     start=True, stop=True)
                    nc.vector.tensor_scalar_add(OF[:, c * 512:(c + 1) * 512], pf[:], bfc_t[:])
                nc.sync.dma_start(out[:], OF[:])

    nc.compile()
    return nc


def run(inputs, trace=False, debug=False):
    key = ("dbg" if debug else "rel")
    if key not in _CACHE:
        _CACHE[key] = _build_nc(debug=debug)
    nc = _CACHE[key]
    in_maps = _prep_host(inputs)
    res = run_bass_kernel_spmd(nc, in_maps, list(range(NCORES)), trace=trace)
    return res


def kernel(**inputs) -> np.ndarray:
    res = run(inputs)
    o = res.results[0]["out"]
    return np.ascontiguousarray(o[0, :N].reshape(N, 1).astype(np.float32))



# revision 7
# speedup vs baseline: 3.5406x; 3.5406x over previous
"""GAT+LSTM fused kernel for 8 trn2 NeuronCores (v2).

- Output depends only on batch row T-1=11 of the reference LSTM (ys[:, -1]),
  so only GAT outputs for nodes [110000, 120000) ("live" nodes) are needed.
- Live node n (0..10239 incl. pad) is owned by core k=n//1280; aggregation
  bucket j=(n%1280)//16, in-bucket row r=16*k + n%16.  With this permutation
  ReduceScatter hands core k partitions [16k,16k+16) = exactly its contiguous
  node range [1280k, 1280k+1280), so LSTM/gx/FC run 8-way sharded.
- Edges sharded by src-range; per-edge src rows (h + a_src, bf16, 264B)
  fetched with one indirect DMA per 128-edge chunk from a per-core blocked
  node table; a_dst comes from an SBUF-resident per-bucket table via a
  transposed one-hot matmul (no second gather).
- Aggregation: one-hot (dst==iota) matmul accumulates each bucket's 2 chunks
  into PSUM; partial accumulators combined with a bf16 ReduceScatter.
- LSTM solved per-core on its 1280 columns + 128-column halo (warm-up from
  zeros; truncation error decays in <64 steps) by ITERS Gauss-Seidel passes
  with tensor_tensor_scan for the cell recurrence.  Core 0 has no halo: its
  halo gx is forced to -30 on the f/i/o gates so the state entering column 0
  is exactly (h=0, c=0).
- Matmuls/tables/collectives in bf16 (fp32 PSUM accumulation); score math,
  softmax normalization and the cell scan stay fp32.
"""
import os
import numpy as np
import ml_dtypes

import concourse.bass as bass
import concourse.bacc as bacc
import concourse.tile as tile
from concourse import mybir
from concourse.bass_utils import run_bass_kernel_spmd
from concourse.masks import make_identity
from contextlib import ExitStack

dt = mybir.dt
F32 = dt.float32
BF16 = dt.bfloat16
AF = mybir.ActivationFunctionType
ALU = mybir.AluOpType
bfnp = ml_dtypes.bfloat16

T, N, F_IN = 12, 10000, 64
HEADS, C, HID = 4, 32, 32
E, TN = 1_000_000, 120_000
NCORES = 8
NSH = TN // NCORES              # 15000 nodes per src shard
NBLK = 118                      # main node-table blocks (118*128 = 15104)
NSHP = NBLK * 128
D0 = (T - 1) * N
DN = N
DBLK = 80                       # live-node buckets (80*128 = 10240)
DNP = DBLK * 128
SL = DNP // NCORES              # 1280 contiguous live nodes owned per core
SBLK = SL // 128                # appended self-node blocks (10*128 = 1280)
NROWS = NSHP + SL               # 16384 rows in h tables
BSLOT = 256                     # canonical slots per bucket (2 chunks)
NCH = DBLK * 2                  # 160 chunks
CAP = NCH * 128                 # 20480
NGRP = 4                        # edge phase pipelined in 4 groups
GC = NCH // NGRP                # 40 chunks per group
HALO = 128
NCOL = HALO + SL                # 1408 LSTM columns per core
ITERS = 5
LEAK = 0.2
W132 = HEADS * C + HEADS        # 132 table row width

_CACHE = {}


def _b16(a):
    return np.ascontiguousarray(np.asarray(a, np.float32).astype(bfnp))


def _chunkify(vals, cap, fill):
    out = np.full(cap, fill, np.float32)
    out[:len(vals)] = vals
    return np.ascontiguousarray(out.reshape(cap // 128, 128).T)


def _row_main(n):
    return (n % 128) * NBLK + n // 128


def _row_self(c):
    return NSHP + (c % 128) * SBLK + c // 128


def _prep_host(inputs):
    x = np.ascontiguousarray(np.asarray(inputs["x_seq"], np.float32).reshape(TN, F_IN))
    ei = np.asarray(inputs["edge_index"])
    ea = np.asarray(inputs["edge_attr"], np.float32)[:, 0]
    W_gat = np.asarray(inputs["W_gat"], np.float32)
    att_src = np.asarray(inputs["att_src"], np.float32)
    att_dst = np.asarray(inputs["att_dst"], np.float32)
    att_edge = np.asarray(inputs["att_edge"], np.float32)
    W_edge = np.asarray(inputs["W_edge"], np.float32)
    gat_bias = np.asarray(inputs["gat_bias"], np.float32)
    W_ih = np.asarray(inputs["W_ih"], np.float32)
    W_hh = np.asarray(inputs["W_hh"], np.float32)
    b = np.asarray(inputs["b_ih"], np.float32) + np.asarray(inputs["b_hh"], np.float32)
    W_fc = np.asarray(inputs["W_fc"], np.float32)
    b_fc = np.asarray(inputs["b_fc"], np.float32)

    A_src = np.zeros((HEADS * C, HEADS), np.float32)
    A_dst = np.zeros((HEADS * C, HEADS), np.float32)
    for h in range(HEADS):
        A_src[h * C:(h + 1) * C, h] = att_src[h]
        A_dst[h * C:(h + 1) * C, h] = att_dst[h]
    wgat_ext = np.concatenate([W_gat, W_gat @ A_src], axis=1)      # [64, 132]
    wad = W_gat @ A_dst                                            # [64, 4]
    kap = np.array([np.dot(W_edge[0, h * C:(h + 1) * C], att_edge[h])
                    for h in range(HEADS)], np.float32)
    kap_rep = np.broadcast_to(kap, (128, HEADS)).copy()
    gb132 = np.zeros((128, W132), np.float32)
    gb132[:, 0:128] = gat_bias[None, :]
    iota128 = np.broadcast_to(np.arange(128, dtype=np.float32), (128, 128)).copy()
    perm = np.concatenate([np.arange(32, 64), np.arange(0, 32),
                           np.arange(96, 128), np.arange(64, 96)])
    WihT = np.ascontiguousarray(W_ih[perm].T)
    WhhT = np.ascontiguousarray(W_hh[perm].T)
    br = np.ascontiguousarray(b[perm].reshape(128, 1))

    src, dst = ei[0].astype(np.int64), ei[1].astype(np.int64)
    live = (dst >= D0) & (dst < D0 + DN)
    core_of = src // NSH

    # x for all live nodes in bucket order: col 128*j + r  <->  node(j, r)
    jj, rr = np.meshgrid(np.arange(DBLK), np.arange(128), indexing="ij")
    node_of = 1280 * (rr // 16) + 16 * jj + (rr % 16)              # [DBLK,128]
    xTD = np.zeros((F_IN, DNP), np.float32)
    mvalid = (node_of < DN).ravel()
    xTD[:, mvalid] = x[D0 + node_of.ravel()[mvalid]].T
    xTD = _b16(xTD)

    ZN = NSH                       # zero-content pad node in src shard
    in_maps = []
    ea_all = ea.reshape(NCORES, E // NCORES)
    for k in range(NCORES):
        m = live & (core_of == k)
        sL = src[m] - k * NSH
        dL = dst[m] - D0
        eav = ea[m]
        bkt = (dL % 1280) // 16
        rowv = 16 * (dL // 1280) + dL % 16
        # own (self-loop) nodes: n = 1280k + c
        own_n = 1280 * k + np.arange(SL)
        own_b = (own_n % 1280) // 16
        own_r = 16 * k + own_n % 16
        hs_idx = np.full(CAP, _row_main(ZN), np.int64)
        eac = np.zeros(CAP, np.float32)
        dstf = np.full(CAP, -1.0, np.float32)
        selfm = np.zeros(CAP, np.float32)
        for bb in range(DBLK):
            sel = np.nonzero(bkt == bb)[0]
            so = np.nonzero(own_b == bb)[0]
            nb = len(sel) + len(so)
            assert nb <= BSLOT, f"core {k} bucket {bb}: {nb} > {BSLOT}"
            o = bb * BSLOT
            ne = len(sel)
            hs_idx[o:o + ne] = _row_main(sL[sel])
            dstf[o:o + ne] = rowv[sel]
            eac[o:o + ne] = eav[sel]
            hs_idx[o + ne:o + nb] = _row_self(so)
            dstf[o + ne:o + nb] = own_r[so]
            selfm[o + ne:o + nb] = 1.0
        xT = np.zeros((F_IN, NSHP), np.float32)
        xT[:, :NSH] = x[k * NSH:(k + 1) * NSH].T
        xTDS = np.zeros((F_IN, SL), np.float32)
        nown = own_n[own_n < DN]
        xTDS[:, :len(nown)] = x[D0 + nown].T
        eaF = np.zeros((128, 980), np.float32)
        ch = ea_all[k]
        j2 = np.arange(len(ch))
        eaF[j2 % 128, j2 // 128] = ch
        halo_idx = ((k - 1) % NCORES) * 128 + np.arange(128)
        halo_mask = np.full((128, 1), 0.0 if k == 0 else 1.0, np.float32)
        halo_fix = np.zeros((128, HALO), np.float32)
        if k == 0:
            halo_fix[0:96, :] = -30.0
        in_maps.append({
            "xT": _b16(xT), "xTD": xTD, "xTDS": _b16(xTDS),
            "eaC": _chunkify(eac, CAP, 0.0),
            "dstF": _chunkify(dstf, CAP, -1.0),
            "selfM": _chunkify(selfm, CAP, 0.0),
            "eaF": eaF,
            "hsI": np.ascontiguousarray(
                hs_idx.reshape(NCH, 128).T).astype(np.int32),
            "WgatE": _b16(wgat_ext), "Wad": _b16(wad),
            "kap": kap_rep, "gb132": gb132, "iota": iota128,
            "Wih": _b16(WihT), "Whh": _b16(WhhT), "br": br,
            "Wfc": _b16(W_fc.reshape(HID, 1)),
            "bfc": np.ascontiguousarray(b_fc.reshape(1, 1)),
            "haloI": np.ascontiguousarray(halo_idx.reshape(128, 1)).astype(np.int32),
            "haloM": halo_mask, "haloF": halo_fix,
        })
    return in_maps


def _build_nc(debug=False):
    STAGE = int(os.environ.get("KSTAGE", "99"))
    nc = bacc.Bacc("TRN2", target_bir_lowering=False, debug=False,
                   num_devices=NCORES)
    g = lambda n, s, d=F32: nc.dram_tensor(n, s, d, kind="ExternalInput").ap()
    xT = g("xT", [F_IN, NSHP], BF16)
    xTD = g("xTD", [F_IN, DNP], BF16)
    xTDS = g("xTDS", [F_IN, SL], BF16)
    eaC = g("eaC", [128, NCH]); dstF = g("dstF", [128, NCH])
    selfM = g("selfM", [128, NCH]); eaF = g("eaF", [128, 980])
    hsI = g("hsI", [128, NCH], dt.int32)
    WgatE = g("WgatE", [F_IN, W132], BF16)
    Wad = g("Wad", [F_IN, HEADS], BF16)
    kap = g("kap", [128, HEADS]); gb132 = g("gb132", [128, W132])
    iota = g("iota", [128, 128])
    Wih = g("Wih", [128, 128], BF16); Whh = g("Whh", [HID, 128], BF16)
    br = g("br", [128, 1]); Wfc = g("Wfc", [HID, 1], BF16); bfc = g("bfc", [1, 1])
    haloI = g("haloI", [128, 1], dt.int32)
    haloM = g("haloM", [128, 1]); haloF = g("haloF", [128, HALO])
    out = nc.dram_tensor("out", [1, SL], F32, kind="ExternalOutput").ap()
    if debug:
        dbg_gat = nc.dram_tensor("dbg_gat", [128, NCOL], F32, kind="ExternalOutput").ap()
        dbg_gx = nc.dram_tensor("dbg_gx", [128, NCOL], F32, kind="ExternalOutput").ap()
        dbg_h = nc.dram_tensor("dbg_h", [HID, NCOL], F32, kind="ExternalOutput").ap()

    h_tbl = nc.dram_tensor("h_tbl", [NROWS, W132], BF16).ap()
    acc_tbl = nc.dram_tensor("acc_tbl", [128, DBLK * W132], BF16).ap()
    rs_out = nc.dram_tensor("rs_out", [16, DBLK * W132], BF16).ap()
    norm_tbl = nc.dram_tensor("norm_tbl", [16, DBLK * W132], F32).ap()
    ea_in = nc.dram_tensor("ea_in", [128, 1], F32).ap()
    ea_out = nc.dram_tensor("ea_out", [128, 1], F32, addr_space="Shared").ap()
    tail_blob = nc.dram_tensor("tail_blob", [128, HALO], BF16).ap()
    tails_full = nc.dram_tensor("tails_full", [NCORES * 128, HALO], BF16,
                                addr_space="Shared").ap()
    RG = [list(range(NCORES))]

    with tile.TileContext(nc) as tc, ExitStack() as top:
        const = top.enter_context(tc.tile_pool(name="const", bufs=1))
        ident = const.tile([128, 128], F32)
        make_identity(nc, ident[:])
        identb = const.tile([128, 128], BF16)
        make_identity(nc, identb[:])
        kap_t = const.tile([128, HEADS], F32); nc.sync.dma_start(kap_t[:], kap[:])
        gb_t = const.tile([128, W132], F32); nc.sync.dma_start(gb_t[:], gb132[:])
        iota_t = const.tile([128, 128], F32); nc.sync.dma_start(iota_t[:], iota[:])
        whh_t = const.tile([HID, 128], BF16); nc.sync.dma_start(whh_t[:], Whh[:])
        wih_t = const.tile([128, 128], BF16); nc.sync.dma_start(wih_t[:], Wih[:])
        br_t = const.tile([128, 1], F32); nc.sync.dma_start(br_t[:], br[:])
        wfc_t = const.tile([HID, 1], BF16); nc.sync.dma_start(wfc_t[:], Wfc[:])
        bfc_t = const.tile([1, 1], F32); nc.sync.dma_start(bfc_t[:], bfc[:])
        haloI_t = const.tile([128, 1], dt.int32); nc.sync.dma_start(haloI_t[:], haloI[:])
        haloM_t = const.tile([128, 1], F32); nc.sync.dma_start(haloM_t[:], haloM[:])
        haloF_t = const.tile([128, HALO], F32); nc.sync.dma_start(haloF_t[:], haloF[:])
        meanr = const.tile([128, 1], F32)
        adS = const.tile([128, DBLK * HEADS], BF16)

        # persistent LSTM-side tiles
        persist = top.enter_context(tc.tile_pool(name="persist", bufs=1))
        GEXT = persist.tile([128, NCOL], BF16)
        gx = persist.tile([128, NCOL], F32)
        H = persist.tile([HID, NCOL + 1], BF16)
        nc.gpsimd.memset(H[:], 0.0)
        if STAGE < 4:
            nc.gpsimd.memset(GEXT[:], 0.0)
        if STAGE < 5:
            nc.gpsimd.memset(gx[:], 0.0)

        # ---------- A1: mean(edge_attr) via AllReduce + PE broadcast -------
        with ExitStack() as ph:
            sbm = ph.enter_context(tc.tile_pool(name="sbm", bufs=1))
            psm = ph.enter_context(tc.tile_pool(name="psm", bufs=1, space="PSUM"))
            eaf_t = sbm.tile([128, 980], F32)
            nc.sync.dma_start(eaf_t[:], eaF[:])
            eap = sbm.tile([128, 1], F32)
            nc.vector.tensor_reduce(eap[:], eaf_t[:], mybir.AxisListType.X, ALU.add)
            nc.sync.dma_start(ea_in[:], eap[:])
            nc.gpsimd.collective_compute("AllReduce", ALU.add, replica_groups=RG,
                                         ins=[ea_in[:]], outs=[ea_out[:]])
            eao_t = sbm.tile([128, 1], F32)
            nc.sync.dma_start(eao_t[:], ea_out[:])
            onc = sbm.tile([128, 1], F32)
            nc.gpsimd.memset(onc[:], 1.0)
            ps1 = psm.tile([1, 1], F32, space="PSUM", tag="ps1")
            nc.tensor.matmul(ps1[:], lhsT=eao_t[:], rhs=onc[:], start=True, stop=True)
            eas = sbm.tile([1, 1], F32)
            nc.scalar.mul(eas[:], ps1[:], 1.0 / E)
            onr = sbm.tile([1, 128], F32)
            nc.gpsimd.memset(onr[:], 1.0)
            ps2 = psm.tile([128, 1], F32, space="PSUM", tag="ps2")
            nc.tensor.matmul(ps2[:], lhsT=onr[:], rhs=eas[:], start=True, stop=True)
            nc.vector.tensor_copy(meanr[:], ps2[:])

        if STAGE >= 1:
            # ---------- A2: blocked node tables (h + a_src), bf16 ----------
            with ExitStack() as ph:
                sba = ph.enter_context(tc.tile_pool(name="sba", bufs=1))
                psa = ph.enter_context(tc.tile_pool(name="psa", bufs=3, space="PSUM"))
                psb = ph.enter_context(tc.tile_pool(name="psb", bufs=2, space="PSUM"))
                psc = ph.enter_context(tc.tile_pool(name="psc", bufs=2, space="PSUM"))
                wg_t = sba.tile([F_IN, W132], BF16)
                nc.sync.dma_start(wg_t[:], WgatE[:])
                wad_t = sba.tile([F_IN, HEADS], BF16)
                nc.sync.dma_start(wad_t[:], Wad[:])
                xt_t = sba.tile([F_IN, NSHP], BF16)
                nc.sync.dma_start(xt_t[:], xT[:])
                HS = sba.tile([128, NBLK * W132], BF16)
                HSv = HS[:].rearrange("p (j w) -> p j w", w=W132)
                for j in range(NBLK):
                    ph_ = psa.tile([128, W132], F32, space="PSUM", tag="ph")
                    nc.tensor.matmul(ph_[:], lhsT=xt_t[:, j * 128:(j + 1) * 128],
                                     rhs=wg_t[:], start=True, stop=True)
                    nc.vector.tensor_copy(HSv[:, j, :], ph_[:])
                nc.sync.dma_start(h_tbl[0:NSHP, :].rearrange("(p j) w -> p (j w)", p=128),
                                  HS[:])
                # appended self-node rows (own slice, plain order)
                xts_t = sba.tile([F_IN, SL], BF16)
                nc.sync.dma_start(xts_t[:], xTDS[:])
                HS2 = sba.tile([128, SBLK * W132], BF16)
                HS2v = HS2[:].rearrange("p (j w) -> p j w", w=W132)
                for j in range(SBLK):
                    ph2 = psb.tile([128, W132], F32, space="PSUM", tag="ph2")
                    nc.tensor.matmul(ph2[:], lhsT=xts_t[:, j * 128:(j + 1) * 128],
                                     rhs=wg_t[:], start=True, stop=True)
                    nc.vector.tensor_copy(HS2v[:, j, :], ph2[:])
                nc.sync.dma_start(
                    h_tbl[NSHP:NROWS, :].rearrange("(p j) w -> p (j w)", p=128), HS2[:])
                # a_dst table for all live nodes, bucket order -> SBUF resident
                xtd_t = sba.tile([F_IN, DNP], BF16)
                nc.sync.dma_start(xtd_t[:], xTD[:])
                for half in range(2):
                    pad_ = psc.tile([128, DBLK * HEADS // 2], F32, space="PSUM",
                                    tag="pad")
                    for j2 in range(DBLK // 2):
                        j = half * (DBLK // 2) + j2
                        nc.tensor.matmul(pad_[:, j2 * HEADS:(j2 + 1) * HEADS],
                                         lhsT=xtd_t[:, j * 128:(j + 1) * 128],
                                         rhs=wad_t[:], start=True, stop=True)
                    nc.vector.tensor_copy(
                        adS[:, half * (DBLK * HEADS // 2):(half + 1) * (DBLK * HEADS // 2)],
                        pad_[:])

        if STAGE >= 2:
            # ---------- A5: edge phase, 4 pipelined groups of 40 chunks ----
            with ExitStack() as ph:
                sbe = ph.enter_context(tc.tile_pool(name="sbe", bufs=1))
                sbg = ph.enter_context(tc.tile_pool(name="sbg", bufs=2))
                pst = ph.enter_context(tc.tile_pool(name="pst", bufs=2, space="PSUM"))
                psd = ph.enter_context(tc.tile_pool(name="psd", bufs=2, space="PSUM"))
                pse = ph.enter_context(tc.tile_pool(name="pse", bufs=2, space="PSUM"))
                ACCT = sbe.tile([128, DBLK * W132], BF16)
                NH = sbe.tile([128, NCH * W132], BF16)
                OHB = sbe.tile([128, NCH * 128], BF16)
                NHv = NH[:].rearrange("p (e w) -> p e w", w=W132)
                OHv = OHB[:].rearrange("p (e w) -> p e w", w=128)
                hsI_t = sbe.tile([128, NCH], dt.int32)
                nc.sync.dma_start(hsI_t[:], hsI[:])
                ea_t = sbe.tile([128, NCH], F32)
                nc.sync.dma_start(ea_t[:], eaC[:])
                df_t = sbe.tile([128, NCH], F32)
                nc.sync.dma_start(df_t[:], dstF[:])
                sm_t = sbe.tile([128, NCH], F32)
                nc.sync.dma_start(sm_t[:], selfM[:])
                for gp in range(NGRP):
                    c0 = gp * GC
                    ADP = psd.tile([128, GC * HEADS], F32, space="PSUM", tag="adp")
                    for cc in range(GC):
                        cg = c0 + cc
                        nc.gpsimd.indirect_dma_start(
                            out=NHv[:, cg, :], out_offset=None, in_=h_tbl[:],
                            in_offset=bass.IndirectOffsetOnAxis(
                                ap=hsI_t[:, cg:cg + 1], axis=0))
                        # one-hot: oh[p, d] = (dstF[p] == d)
                        nc.vector.tensor_tensor(
                            out=OHv[:, cg, :],
                            in0=df_t[:, cg:cg + 1].to_broadcast([128, 128]),
                            in1=iota_t[:], op=ALU.is_equal)
                        # transposed one-hot -> per-edge a_dst
                        trp = pst.tile([128, 128], BF16, space="PSUM", tag="trp")
                        nc.tensor.transpose(out=trp[:], in_=OHv[:, cg, :],
                                            identity=identb[:])
                        ohT = sbg.tile([128, 128], BF16, tag="ohT")
                        nc.vector.tensor_copy(ohT[:], trp[:])
                        nc.tensor.matmul(
                            ADP[:, cc * HEADS:(cc + 1) * HEADS], lhsT=ohT[:],
                            rhs=adS[:, (cg // 2) * HEADS:(cg // 2 + 1) * HEADS],
                            start=True, stop=True)
                    # ---- batched score math for this group (fp32) ----
                    ADE = sbg.tile([128, GC * HEADS], F32, tag="ADE")
                    nc.vector.tensor_copy(ADE[:], ADP[:])
                    ASR = sbg.tile([128, GC * HEADS], F32, tag="ASR")
                    nc.vector.tensor_copy(
                        ASR[:].rearrange("p (e w) -> p e w", w=HEADS),
                        NHv[:, c0:c0 + GC, 128:132])
                    EAm = sbg.tile([128, GC], F32, tag="EAm")
                    nc.vector.scalar_tensor_tensor(
                        out=EAm[:], in0=sm_t[:, c0:c0 + GC], scalar=meanr[:],
                        op0=ALU.mult, op1=ALU.add, in1=ea_t[:, c0:c0 + GC])
                    Q = sbg.tile([128, GC * HEADS], F32, tag="Q")
                    Qv = Q[:].rearrange("p (e w) -> p e w", w=HEADS)
                    nc.vector.tensor_tensor(out=Q[:], in0=ASR[:], in1=ADE[:],
                                            op=ALU.add)
                    T2 = sbg.tile([128, GC * HEADS], F32, tag="T2")
                    T2v = T2[:].rearrange("p (e w) -> p e w", w=HEADS)
                    ea3 = EAm[:].rearrange("p (e w) -> p e w", w=1) \
                        .to_broadcast([128, GC, HEADS])
                    kap3 = kap_t[:].rearrange("p (o w) -> p o w", o=1) \
                        .to_broadcast([128, GC, HEADS])
                    nc.vector.tensor_tensor(out=T2v, in0=ea3, in1=kap3, op=ALU.mult)
                    nc.vector.tensor_tensor(out=Q[:], in0=Q[:], in1=T2[:], op=ALU.add)
                    nc.vector.tensor_scalar_mul(T2[:], Q[:], LEAK)
                    nc.vector.tensor_tensor(out=Q[:], in0=Q[:], in1=T2[:], op=ALU.max)
                    # exp(q) = sigmoid(q)/sigmoid(-q); Exp ACT table not resident
                    SG1 = sbg.tile([128, GC * HEADS], F32, tag="SG1")
                    nc.scalar.activation(SG1[:], Q[:], AF.Sigmoid)
                    nc.scalar.activation(T2[:], Q[:], AF.Sigmoid, scale=-1.0)
                    nc.vector.reciprocal(T2[:], T2[:])
                    nc.vector.tensor_tensor(out=Q[:], in0=SG1[:], in1=T2[:],
                                            op=ALU.mult)
                    S4b = sbg.tile([128, GC * HEADS], BF16, tag="S4b")
                    nc.vector.tensor_copy(S4b[:], Q[:])
                    S4v = S4b[:].rearrange("p (e w) -> p e w", w=HEADS)
                    nc.vector.tensor_copy(NHv[:, c0:c0 + GC, 128:132], S4v)
                    nh4 = NHv[:, c0:c0 + GC, 0:128] \
                        .rearrange("p e (h c) -> p e h c", h=HEADS)
                    sc4 = S4v.rearrange("p e (h c) -> p e h c", c=1) \
                        .to_broadcast([128, GC, HEADS, C])
                    nc.vector.tensor_tensor(out=nh4, in0=nh4, in1=sc4, op=ALU.mult)
                    # ---- aggregation: 2 chunks per bucket into PSUM ----
                    for b2 in range(GC // 2):
                        bkt = gp * (GC // 2) + b2
                        pacc = pse.tile([128, W132], F32, space="PSUM", tag="pacc")
                        nc.tensor.matmul(pacc[:], lhsT=OHv[:, 2 * bkt, :],
                                         rhs=NHv[:, 2 * bkt, :], start=True, stop=False)
                        nc.tensor.matmul(pacc[:], lhsT=OHv[:, 2 * bkt + 1, :],
                                         rhs=NHv[:, 2 * bkt + 1, :], start=False,
                                         stop=True)
                        nc.vector.tensor_copy(
                            ACCT[:, bkt * W132:(bkt + 1) * W132], pacc[:])
                nc.sync.dma_start(acc_tbl[:], ACCT[:])

        if STAGE >= 3:
            # ---------- A6: ReduceScatter (bf16) ----------
            nc.gpsimd.collective_compute("ReduceScatter", ALU.add, replica_groups=RG,
                                         ins=[acc_tbl[:]], outs=[rs_out[:]])

            # ---------- A7: normalize + bias + ReLU at full width ----------
            with ExitStack() as ph:
                sbn = ph.enter_context(tc.tile_pool(name="sbn", bufs=1))
                RSS = sbn.tile([128, 10 * W132], BF16)
                nc.sync.dma_start(
                    RSS[:], rs_out[:].rearrange("r (m f) -> (r m) f", f=10 * W132))
                RSF = sbn.tile([128, 10 * W132], F32)
                nc.vector.tensor_copy(RSF[:], RSS[:])
                RSv = RSF[:].rearrange("p (j w) -> p j w", w=W132)
                DEN = sbn.tile([128, 10 * HEADS], F32)
                DENv = DEN[:].rearrange("p (j w) -> p j w", w=HEADS)
                nc.vector.tensor_scalar_add(DENv, RSv[:, :, 128:132], 1e-16)
                nc.vector.reciprocal(DEN[:], DEN[:])
                r4 = DENv.rearrange("p j (h c) -> p j h c", c=1) \
                    .to_broadcast([128, 10, HEADS, C])
                m4 = RSv[:, :, 0:128].rearrange("p j (h c) -> p j h c", h=HEADS)
                nc.vector.tensor_tensor(out=m4, in0=m4, in1=r4, op=ALU.mult)
                gbb = gb_t[:, 0:128].rearrange("p (o w) -> p o w", o=1) \
                    .to_broadcast([128, 10, 128])
                nc.vector.tensor_tensor(out=RSv[:, :, 0:128], in0=RSv[:, :, 0:128],
                                        in1=gbb, op=ALU.add)
                nc.vector.tensor_scalar_max(RSv[:, :, 0:128], RSv[:, :, 0:128], 0.0)
                nc.sync.dma_start(
                    norm_tbl[:].rearrange("r (m f) -> (r m) f", f=10 * W132), RSF[:])

            # ---------- A7b: transpose own slice into GEXT ----------
            with ExitStack() as ph:
                sbt = ph.enter_context(tc.tile_pool(name="sbt", bufs=1))
                psn = ph.enter_context(tc.tile_pool(name="psn", bufs=4, space="PSUM"))
                TRS = sbt.tile([16, DBLK * W132], F32)
                nc.sync.dma_start(TRS[:], norm_tbl[:])
                TRv = TRS[:].rearrange("p (j w) -> p j w", w=W132)
                for j in range(DBLK):
                    ptr = psn.tile([128, 16], F32, space="PSUM", tag="ptr")
                    nc.tensor.transpose(out=ptr[:], in_=TRv[:, j, 0:128],
                                        identity=ident[0:16, 0:16])
                    nc.vector.tensor_copy(GEXT[:, HALO + j * 16:HALO + (j + 1) * 16],
                                          ptr[:])
                nc.sync.dma_start(tail_blob[:], GEXT[:, SL:SL + HALO])

        if STAGE >= 4:
            # ---------- A8: tail AllGather + halo fetch ----------
            nc.gpsimd.collective_compute("AllGather", ALU.bypass, replica_groups=RG,
                                         ins=[tail_blob[:]], outs=[tails_full[:]])
            nc.gpsimd.indirect_dma_start(
                out=GEXT[:, 0:HALO], out_offset=None, in_=tails_full[:],
                in_offset=bass.IndirectOffsetOnAxis(ap=haloI_t[:], axis=0))
        if debug:
            dbgG = persist.tile([128, NCOL], F32)
            nc.vector.tensor_copy(dbgG[:], GEXT[:])
            nc.sync.dma_start(dbg_gat[:], dbgG[:])

        if STAGE >= 5:
            # ---------- A9: gx = W_ih @ gat + b; halo fix ----------
            with ExitStack() as ph:
                psg = ph.enter_context(tc.tile_pool(name="psg", bufs=4, space="PSUM"))
                for c, (lo, hi) in enumerate([(0, 512), (512, 1024), (1024, NCOL)]):
                    pg = psg.tile([128, hi - lo], F32, space="PSUM", tag="pg")
                    nc.tensor.matmul(pg[:], lhsT=wih_t[:], rhs=GEXT[:, lo:hi],
                                     start=True, stop=True)
                    nc.vector.tensor_scalar_add(gx[:, lo:hi], pg[:], br_t[:])
                nc.vector.scalar_tensor_tensor(
                    out=gx[:, 0:HALO], in0=gx[:, 0:HALO], scalar=haloM_t[:],
                    op0=ALU.mult, op1=ALU.add, in1=haloF_t[:])
        if debug:
            nc.sync.dma_start(dbg_gx[:], gx[:])

        if STAGE >= 6:
            # ---------- A10: LSTM fixed point, single 1408-col chain ------
            with ExitStack() as ph:
                sbl = ph.enter_context(tc.tile_pool(name="sbl", bufs=2))
                psl = ph.enter_context(tc.tile_pool(name="psl", bufs=4, space="PSUM"))
                for it in range(ITERS):
                    if it == 0:
                        Gp = gx[:]
                    else:
                        Gs = sbl.tile([128, NCOL], F32, tag="Gs")
                        for lo, hi in [(0, 512), (512, 1024), (1024, NCOL)]:
                            pG = psl.tile([128, hi - lo], F32, space="PSUM", tag="pG")
                            nc.tensor.matmul(pG[:], lhsT=whh_t[:], rhs=H[:, lo:hi],
                                             start=True, stop=True)
                            nc.vector.tensor_tensor(out=Gs[:, lo:hi], in0=pG[:],
                                                    in1=gx[:, lo:hi], op=ALU.add)
                        Gp = Gs[:]
                    S_ = sbl.tile([96, NCOL], F32, tag="S")
                    nc.scalar.activation(S_[:], Gp[0:96, :], AF.Sigmoid)
                    Tg = sbl.tile([64, NCOL], F32, tag="Tg")
                    nc.scalar.activation(Tg[32:64, :], Gp[96:128, :], AF.Tanh)
                    Zt = sbl.tile([HID, NCOL], F32, tag="Zt")
                    nc.vector.tensor_tensor(out=Zt[:], in0=S_[32:64, :],
                                            in1=Tg[32:64, :], op=ALU.mult)
                    Ct = sbl.tile([HID, NCOL], F32, tag="Ct")
                    nc.vector.tensor_tensor_scan(
                        out=Ct[:], data0=S_[0:32, :], data1=Zt[:], initial=0.0,
                        op0=ALU.mult, op1=ALU.add)
                    TC = sbl.tile([96, NCOL], F32, tag="TC")
                    nc.scalar.activation(TC[64:96, :], Ct[:], AF.Tanh)
                    nc.vector.tensor_tensor(out=H[:, 1:NCOL + 1], in0=S_[64:96, :],
                                            in1=TC[64:96, :], op=ALU.mult)
        if debug:
            dbgH = persist.tile([HID, NCOL], F32)
            nc.vector.tensor_copy(dbgH[:], H[:, 1:NCOL + 1])
            nc.sync.dma_start(dbg_h[:], dbgH[:])

        if STAGE >= 7:
            # ---------- FC on own 1280 columns ----------
            with ExitStack() as ph:
                sbf = ph.enter_context(tc.tile_pool(name="sbf", bufs=1))
                psf = ph.enter_context(tc.tile_pool(name="psf", bufs=4, space="PSUM"))
                OF = sbf.tile([1, SL], F32)
                for c, (lo, hi) in enumerate([(0, 512), (512, 1024), (1024, SL)]):
                    pf = psf.tile([1, hi - lo], F32, space="PSUM", tag="pf")
                    nc.tensor.matmul(pf[:], lhsT=wfc_t[:],
                                     rhs=H[:, HALO + 1 + lo:HALO + 1 + hi],
                                     start=True, stop=True)
                    nc.vector.tensor_scalar_add(OF[:, lo:hi], pf[:], bfc_t[:])
                nc.sync.dma_start(out[:], OF[:])

    nc.compile()
    return nc


def run(inputs, trace=False, debug=False):
    key = ("dbg" if debug else "rel")
    if key not in _CACHE:
        _CACHE[key] = _build_nc(debug=debug)
    nc = _CACHE[key]
    in_maps = _prep_host(inputs)
    res = run_bass_kernel_spmd(nc, in_maps, list(range(NCORES)), trace=trace)
    return res


def kernel(**inputs) -> np.ndarray:
    res = run(inputs)
    o = np.concatenate([np.asarray(res.results[k]["out"][0], np.float32)
                        for k in range(NCORES)])
    return np.ascontiguousarray(o[:N].reshape(N, 1))


# revision 18
# speedup vs baseline: 3.7189x; 1.0504x over previous
"""GAT+LSTM fused kernel for 8 trn2 NeuronCores (v2).

- Output depends only on batch row T-1=11 of the reference LSTM (ys[:, -1]),
  so only GAT outputs for nodes [110000, 120000) ("live" nodes) are needed.
- Live node n (0..10239 incl. pad) is owned by core k=n//1280; aggregation
  bucket j=(n%1280)//16, in-bucket row r=16*k + n%16.  With this permutation
  ReduceScatter hands core k partitions [16k,16k+16) = exactly its contiguous
  node range [1280k, 1280k+1280), so LSTM/gx/FC run 8-way sharded.
- Edges sharded by src-range; per-edge src rows (h + a_src, bf16, 264B)
  fetched with one indirect DMA per 128-edge chunk from a per-core blocked
  node table; a_dst comes from an SBUF-resident per-bucket table via a
  transposed one-hot matmul (no second gather).
- Aggregation: one-hot (dst==iota) matmul accumulates each bucket's 2 chunks
  into PSUM; partial accumulators combined with a bf16 ReduceScatter.
- LSTM solved per-core on its 1280 columns + 128-column halo (warm-up from
  zeros; truncation error decays in <64 steps) by ITERS Gauss-Seidel passes
  with tensor_tensor_scan for the cell recurrence.  Core 0 has no halo: its
  halo gx is forced to -30 on the f/i/o gates so the state entering column 0
  is exactly (h=0, c=0).
- Matmuls/tables/collectives in bf16 (fp32 PSUM accumulation); score math,
  softmax normalization and the cell scan stay fp32.
"""
import os
import numpy as np
import ml_dtypes

import concourse.bass as bass
import concourse.bacc as bacc
import concourse.tile as tile
from concourse import mybir
from concourse.bass_utils import run_bass_kernel_spmd
from concourse.masks import make_identity
from contextlib import ExitStack

dt = mybir.dt
F32 = dt.float32
BF16 = dt.bfloat16
AF = mybir.ActivationFunctionType
ALU = mybir.AluOpType
bfnp = ml_dtypes.bfloat16

T, N, F_IN = 12, 10000, 64
HEADS, C, HID = 4, 32, 32
E, TN = 1_000_000, 120_000
NCORES = 8
NSH = TN // NCORES              # 15000 nodes per src shard
NBLK = 118                      # main node-table blocks (118*128 = 15104)
NSHP = NBLK * 128
D0 = (T - 1) * N
DN = N
DBLK = 80                       # live-node buckets (80*128 = 10240)
DNP = DBLK * 128
SL = DNP // NCORES              # 1280 contiguous live nodes owned per core
SBLK = SL // 128                # appended self-node blocks (10*128 = 1280)
NROWS = NSHP + SL               # 16384 rows in h tables
BSLOT = 256                     # canonical slots per bucket (2 chunks)
NCH = DBLK * 2                  # 160 chunks
CAP = NCH * 128                 # 20480
NGRP = 4                        # edge phase pipelined in 4 groups
GC = NCH // NGRP                # 40 chunks per group
HALO = 128
NCOL = HALO + SL                # 1408 LSTM columns per core
ITERS = 5
LEAK = 0.2
W132 = HEADS * C + HEADS        # 132 table row width

_CACHE = {}


def _b16(a):
    return np.ascontiguousarray(np.asarray(a, np.float32).astype(bfnp))


def _chunkify(vals, cap, fill):
    out = np.full(cap, fill, np.float32)
    out[:len(vals)] = vals
    return np.ascontiguousarray(out.reshape(cap // 128, 128).T)


def _row_main(n):
    return (n % 128) * NBLK + n // 128


def _row_self(c):
    return NSHP + (c % 128) * SBLK + c // 128


def _prep_host(inputs):
    x = np.ascontiguousarray(np.asarray(inputs["x_seq"], np.float32).reshape(TN, F_IN))
    ei = np.asarray(inputs["edge_index"])
    ea = np.asarray(inputs["edge_attr"], np.float32)[:, 0]
    W_gat = np.asarray(inputs["W_gat"], np.float32)
    att_src = np.asarray(inputs["att_src"], np.float32)
    att_dst = np.asarray(inputs["att_dst"], np.float32)
    att_edge = np.asarray(inputs["att_edge"], np.float32)
    W_edge = np.asarray(inputs["W_edge"], np.float32)
    gat_bias = np.asarray(inputs["gat_bias"], np.float32)
    W_ih = np.asarray(inputs["W_ih"], np.float32)
    W_hh = np.asarray(inputs["W_hh"], np.float32)
    b = np.asarray(inputs["b_ih"], np.float32) + np.asarray(inputs["b_hh"], np.float32)
    W_fc = np.asarray(inputs["W_fc"], np.float32)
    b_fc = np.asarray(inputs["b_fc"], np.float32)

    A_src = np.zeros((HEADS * C, HEADS), np.float32)
    A_dst = np.zeros((HEADS * C, HEADS), np.float32)
    for h in range(HEADS):
        A_src[h * C:(h + 1) * C, h] = att_src[h]
        A_dst[h * C:(h + 1) * C, h] = att_dst[h]
    wgat_ext = np.concatenate([W_gat, W_gat @ A_src], axis=1)      # [64, 132]
    wad = W_gat @ A_dst                                            # [64, 4]
    kap = np.array([np.dot(W_edge[0, h * C:(h + 1) * C], att_edge[h])
                    for h in range(HEADS)], np.float32)
    kap_rep = np.broadcast_to(kap, (128, HEADS)).copy()
    gb132 = np.zeros((128, W132), np.float32)
    gb132[:, 0:128] = gat_bias[None, :]
    iota128 = np.broadcast_to(np.arange(128, dtype=np.float32), (128, 128)).copy()
    perm = np.concatenate([np.arange(32, 64), np.arange(0, 32),
                           np.arange(96, 128), np.arange(64, 96)])
    WihT = np.ascontiguousarray(W_ih[perm].T)
    WhhT = np.ascontiguousarray(W_hh[perm].T)
    br = np.ascontiguousarray(b[perm].reshape(128, 1))

    src, dst = ei[0].astype(np.int64), ei[1].astype(np.int64)
    live = (dst >= D0) & (dst < D0 + DN)
    core_of = src // NSH

    # x for all live nodes in bucket order: col 128*j + r  <->  node(j, r)
    jj, rr = np.meshgrid(np.arange(DBLK), np.arange(128), indexing="ij")
    node_of = 1280 * (rr // 16) + 16 * jj + (rr % 16)              # [DBLK,128]
    xTD = np.zeros((F_IN, DNP), np.float32)
    mvalid = (node_of < DN).ravel()
    xTD[:, mvalid] = x[D0 + node_of.ravel()[mvalid]].T
    xTD = _b16(xTD)

    ZN = NSH                       # zero-content pad node in src shard
    in_maps = []
    EAW = (E + 127) // 128         # 7813
    eaFull = np.zeros((128, EAW), np.float32)
    eaFull.ravel()[:E] = ea        # row-major fill; order irrelevant for sum
    eaFull = _b16(eaFull)
    iotaP = np.ascontiguousarray(np.arange(128, dtype=np.float32).reshape(128, 1))
    for k in range(NCORES):
        m = live & (core_of == k)
        sL = src[m] - k * NSH
        dL = dst[m] - D0
        eav = ea[m]
        bkt = (dL % 1280) // 16
        rowv = 16 * (dL // 1280) + dL % 16
        # own (self-loop) nodes: n = 1280k + c
        own_n = 1280 * k + np.arange(SL)
        own_b = (own_n % 1280) // 16
        own_r = 16 * k + own_n % 16
        hs_idx = np.full(CAP, _row_main(ZN), np.int64)
        eac = np.zeros(CAP, np.float32)
        dstf = np.full(CAP, -1.0, np.float32)
        selfm = np.zeros(CAP, np.float32)
        for bb in range(DBLK):
            sel = np.nonzero(bkt == bb)[0]
            so = np.nonzero(own_b == bb)[0]
            nb = len(sel) + len(so)
            assert nb <= BSLOT, f"core {k} bucket {bb}: {nb} > {BSLOT}"
            o = bb * BSLOT
            ne = len(sel)
            hs_idx[o:o + ne] = _row_main(sL[sel])
            dstf[o:o + ne] = rowv[sel]
            eac[o:o + ne] = eav[sel]
            hs_idx[o + ne:o + nb] = _row_self(so)
            dstf[o + ne:o + nb] = own_r[so]
            selfm[o + ne:o + nb] = 1.0
        xT = np.zeros((F_IN, NSHP), np.float32)
        xT[:, :NSH] = x[k * NSH:(k + 1) * NSH].T
        xTDS = np.zeros((F_IN, SL), np.float32)
        nown = own_n[own_n < DN]
        xTDS[:, :len(nown)] = x[D0 + nown].T
        halo_idx = ((k - 1) % NCORES) * 128 + np.arange(128)
        halo_mask = np.full((128, 1), 0.0 if k == 0 else 1.0, np.float32)
        halo_fix = np.zeros((128, HALO), np.float32)
        if k == 0:
            halo_fix[0:96, :] = -30.0
        in_maps.append({
            "xT": _b16(xT), "xTD": xTD, "xTDS": _b16(xTDS),
            "eaC": _chunkify(eac, CAP, 0.0),
            "dstF": _chunkify(dstf, CAP, -1.0),
            "dfFlat": _b16(dstf.reshape(1, CAP)),
            "selfM": _chunkify(selfm, CAP, 0.0),
            "eaFull": eaFull,
            "iotaP": iotaP,
            "hsI": np.ascontiguousarray(
                hs_idx.reshape(NCH, 128).T).astype(np.int32),
            "WgatE": _b16(wgat_ext), "Wad": _b16(wad),
            "kap": kap_rep, "gb132": gb132, "iota": iota128,
            "Wih": _b16(WihT), "Whh": _b16(WhhT), "br": br,
            "Wfc": _b16(W_fc.reshape(HID, 1)),
            "bfc": np.ascontiguousarray(b_fc.reshape(1, 1)),
            "haloI": np.ascontiguousarray(halo_idx.reshape(128, 1)).astype(np.int32),
            "haloM": halo_mask, "haloF": halo_fix,
        })
    return in_maps


def _build_nc(debug=False):
    STAGE = int(os.environ.get("KSTAGE", "99"))
    nc = bacc.Bacc("TRN2", target_bir_lowering=False, debug=False,
                   num_devices=NCORES)
    g = lambda n, s, d=F32: nc.dram_tensor(n, s, d, kind="ExternalInput").ap()
    xT = g("xT", [F_IN, NSHP], BF16)
    xTD = g("xTD", [F_IN, DNP], BF16)
    xTDS = g("xTDS", [F_IN, SL], BF16)
    eaC = g("eaC", [128, NCH]); dstF = g("dstF", [128, NCH])
    dfFlat = g("dfFlat", [1, CAP], BF16)
    selfM = g("selfM", [128, NCH])
    EAW = (E + 127) // 128
    eaFull = g("eaFull", [128, EAW], BF16)
    iotaP = g("iotaP", [128, 1])
    hsI = g("hsI", [128, NCH], dt.int32)
    WgatE = g("WgatE", [F_IN, W132], BF16)
    Wad = g("Wad", [F_IN, HEADS], BF16)
    kap = g("kap", [128, HEADS]); gb132 = g("gb132", [128, W132])
    iota = g("iota", [128, 128])
    Wih = g("Wih", [128, 128], BF16); Whh = g("Whh", [HID, 128], BF16)
    br = g("br", [128, 1]); Wfc = g("Wfc", [HID, 1], BF16); bfc = g("bfc", [1, 1])
    haloI = g("haloI", [128, 1], dt.int32)
    haloM = g("haloM", [128, 1]); haloF = g("haloF", [128, HALO])
    out = nc.dram_tensor("out", [1, SL], F32, kind="ExternalOutput").ap()
    if debug:
        dbg_gat = nc.dram_tensor("dbg_gat", [128, NCOL], F32, kind="ExternalOutput").ap()
        dbg_gx = nc.dram_tensor("dbg_gx", [128, NCOL], F32, kind="ExternalOutput").ap()
        dbg_h = nc.dram_tensor("dbg_h", [HID, NCOL], F32, kind="ExternalOutput").ap()

    h_tbl = nc.dram_tensor("h_tbl", [NROWS, W132], BF16).ap()
    acc_tbl = nc.dram_tensor("acc_tbl", [128, DBLK * W132], BF16).ap()
    rs_out = nc.dram_tensor("rs_out", [16, DBLK * W132], BF16).ap()
    norm_tbl = nc.dram_tensor("norm_tbl", [16, DBLK * W132], F32).ap()
    tail_blob = nc.dram_tensor("tail_blob", [128, HALO], BF16).ap()
    tails_full = nc.dram_tensor("tails_full", [NCORES * 128, HALO], BF16,
                                addr_space="Shared").ap()
    RG = [list(range(NCORES))]

    with tile.TileContext(nc) as tc, ExitStack() as top:
        const = top.enter_context(tc.tile_pool(name="const", bufs=1))
        ident = const.tile([128, 128], F32)
        make_identity(nc, ident[:])
        iotaP_t = const.tile([128, 1], F32); nc.sync.dma_start(iotaP_t[:], iotaP[:])
        ones1 = const.tile([1, 128], BF16)
        nc.gpsimd.memset(ones1[:], 1.0)
        kap_t = const.tile([128, HEADS], F32); nc.sync.dma_start(kap_t[:], kap[:])
        gb_t = const.tile([128, W132], F32); nc.sync.dma_start(gb_t[:], gb132[:])
        iota_t = const.tile([128, 128], F32); nc.sync.dma_start(iota_t[:], iota[:])
        whh_t = const.tile([HID, 128], BF16); nc.sync.dma_start(whh_t[:], Whh[:])
        wih_t = const.tile([128, 128], BF16); nc.sync.dma_start(wih_t[:], Wih[:])
        br_t = const.tile([128, 1], F32); nc.sync.dma_start(br_t[:], br[:])
        wfc_t = const.tile([HID, 1], BF16); nc.sync.dma_start(wfc_t[:], Wfc[:])
        bfc_t = const.tile([1, 1], F32); nc.sync.dma_start(bfc_t[:], bfc[:])
        haloI_t = const.tile([128, 1], dt.int32); nc.sync.dma_start(haloI_t[:], haloI[:])
        haloM_t = const.tile([128, 1], F32); nc.sync.dma_start(haloM_t[:], haloM[:])
        haloF_t = const.tile([128, HALO], F32); nc.sync.dma_start(haloF_t[:], haloF[:])
        meanr = const.tile([128, 1], F32)
        adS = const.tile([128, DBLK * HEADS], BF16)

        # persistent LSTM-side tiles
        persist = top.enter_context(tc.tile_pool(name="persist", bufs=1))
        GEXT = persist.tile([128, NCOL], BF16)
        gx = persist.tile([128, NCOL], F32)
        H = persist.tile([HID, NCOL + 1], BF16)
        nc.gpsimd.memset(H[:], 0.0)
        if STAGE < 4:
            nc.gpsimd.memset(GEXT[:], 0.0)
        if STAGE < 5:
            nc.gpsimd.memset(gx[:], 0.0)

        # ---------- A1: mean(edge_attr) — full copy per core, no collective
        with ExitStack() as ph:
            sbm = ph.enter_context(tc.tile_pool(name="sbm", bufs=1))
            psm = ph.enter_context(tc.tile_pool(name="psm", bufs=1, space="PSUM"))
            eaf_t = sbm.tile([128, EAW], BF16)
            nc.sync.dma_start(eaf_t[:], eaFull[:])
            eap = sbm.tile([128, 1], F32)
            nc.vector.tensor_reduce(eap[:], eaf_t[:], mybir.AxisListType.X, ALU.add)
            onc = sbm.tile([128, 1], F32)
            nc.gpsimd.memset(onc[:], 1.0)
            ps1 = psm.tile([1, 1], F32, space="PSUM", tag="ps1")
            nc.tensor.matmul(ps1[:], lhsT=eap[:], rhs=onc[:], start=True, stop=True)
            eas = sbm.tile([1, 1], F32)
            nc.scalar.mul(eas[:], ps1[:], 1.0 / E)
            onr = sbm.tile([1, 128], F32)
            nc.gpsimd.memset(onr[:], 1.0)
            ps2 = psm.tile([128, 1], F32, space="PSUM", tag="ps2")
            nc.tensor.matmul(ps2[:], lhsT=onr[:], rhs=eas[:], start=True, stop=True)
            nc.vector.tensor_copy(meanr[:], ps2[:])

        if STAGE >= 1:
            # ---------- A2: blocked node tables (h + a_src), bf16 ----------
            with ExitStack() as ph:
                sba = ph.enter_context(tc.tile_pool(name="sba", bufs=1))
                psa = ph.enter_context(tc.tile_pool(name="psa", bufs=3, space="PSUM"))
                psb = ph.enter_context(tc.tile_pool(name="psb", bufs=2, space="PSUM"))
                psc = ph.enter_context(tc.tile_pool(name="psc", bufs=2, space="PSUM"))
                xt_t = sba.tile([F_IN, NSHP], BF16)
                nc.sync.dma_start(xt_t[:], xT[:])
                wg_t = sba.tile([F_IN, W132], BF16)
                nc.sync.dma_start(wg_t[:], WgatE[:])
                wad_t = sba.tile([F_IN, HEADS], BF16)
                nc.sync.dma_start(wad_t[:], Wad[:])
                HS = sba.tile([128, NBLK * W132], BF16)
                HSv = HS[:].rearrange("p (j w) -> p j w", w=W132)
                for j0 in range(0, NBLK, 3):
                    nb = min(3, NBLK - j0)
                    ph_ = psa.tile([128, 3 * W132], F32, space="PSUM", tag="ph")
                    for j in range(j0, j0 + nb):
                        nc.tensor.matmul(ph_[:, (j - j0) * W132:(j - j0 + 1) * W132],
                                         lhsT=xt_t[:, j * 128:(j + 1) * 128],
                                         rhs=wg_t[:], start=True, stop=True)
                    nc.vector.tensor_copy(
                        HS[:, j0 * W132:(j0 + nb) * W132], ph_[:, 0:nb * W132])
                nc.sync.dma_start(h_tbl[0:NSHP, :].rearrange("(p j) w -> p (j w)", p=128),
                                  HS[:])
                # appended self-node rows (own slice, plain order)
                xts_t = sba.tile([F_IN, SL], BF16)
                nc.sync.dma_start(xts_t[:], xTDS[:])
                HS2 = sba.tile([128, SBLK * W132], BF16)
                for j0 in range(0, SBLK, 3):
                    nb = min(3, SBLK - j0)
                    ph2 = psb.tile([128, 3 * W132], F32, space="PSUM", tag="ph2")
                    for j in range(j0, j0 + nb):
                        nc.tensor.matmul(ph2[:, (j - j0) * W132:(j - j0 + 1) * W132],
                                         lhsT=xts_t[:, j * 128:(j + 1) * 128],
                                         rhs=wg_t[:], start=True, stop=True)
                    nc.vector.tensor_copy(
                        HS2[:, j0 * W132:(j0 + nb) * W132], ph2[:, 0:nb * W132])
                nc.sync.dma_start(
                    h_tbl[NSHP:NROWS, :].rearrange("(p j) w -> p (j w)", p=128), HS2[:])
                # a_dst table for all live nodes, bucket order -> SBUF resident
                xtd_t = sba.tile([F_IN, DNP], BF16)
                nc.sync.dma_start(xtd_t[:], xTD[:])
                for half in range(2):
                    pad_ = psc.tile([128, DBLK * HEADS // 2], F32, space="PSUM",
                                    tag="pad")
                    for j2 in range(DBLK // 2):
                        j = half * (DBLK // 2) + j2
                        nc.tensor.matmul(pad_[:, j2 * HEADS:(j2 + 1) * HEADS],
                                         lhsT=xtd_t[:, j * 128:(j + 1) * 128],
                                         rhs=wad_t[:], start=True, stop=True)
                    nc.vector.tensor_copy(
                        adS[:, half * (DBLK * HEADS // 2):(half + 1) * (DBLK * HEADS // 2)],
                        pad_[:])

        if STAGE >= 2:
            # ---------- A5: edge phase, 4 pipelined groups of 40 chunks ----
            with ExitStack() as ph:
                sbe = ph.enter_context(tc.tile_pool(name="sbe", bufs=1))
                sbg = ph.enter_context(tc.tile_pool(name="sbg", bufs=2))
                pst = ph.enter_context(tc.tile_pool(name="pst", bufs=2, space="PSUM"))
                psd = ph.enter_context(tc.tile_pool(name="psd", bufs=2, space="PSUM"))
                pse = ph.enter_context(tc.tile_pool(name="pse", bufs=2, space="PSUM"))
                ACCT = sbe.tile([128, DBLK * W132], BF16)
                NH = sbe.tile([128, NCH * W132], BF16)
                OHB = sbe.tile([128, NCH * 128], BF16)
                NHv = NH[:].rearrange("p (e w) -> p e w", w=W132)
                OHv = OHB[:].rearrange("p (e w) -> p e w", w=128)
                hsI_t = sbe.tile([128, NCH], dt.int32)
                nc.sync.dma_start(hsI_t[:], hsI[:])
                ea_t = sbe.tile([128, NCH], F32)
                nc.sync.dma_start(ea_t[:], eaC[:])
                df_t = sbe.tile([128, NCH], F32)
                nc.sync.dma_start(df_t[:], dstF[:])
                dfF_t = sbe.tile([1, CAP], BF16)
                nc.sync.dma_start(dfF_t[:], dfFlat[:])
                sm_t = sbe.tile([128, NCH], F32)
                nc.sync.dma_start(sm_t[:], selfM[:])
                for gp in range(NGRP):
                    c0 = gp * GC
                    for cc in range(GC):
                        cg = c0 + cc
                        nc.gpsimd.indirect_dma_start(
                            out=NHv[:, cg, :], out_offset=None, in_=h_tbl[:],
                            in_offset=bass.IndirectOffsetOnAxis(
                                ap=hsI_t[:, cg:cg + 1], axis=0))
                    # one-hot for the whole group: oh[p, (e d)] = (dstF[p,e] == d)
                    nc.vector.tensor_tensor(
                        out=OHv[:, c0:c0 + GC, :],
                        in0=df_t[:, c0:c0 + GC].rearrange("p (e w) -> p e w", w=1)
                        .to_broadcast([128, GC, 128]),
                        in1=iota_t[:].rearrange("p (o w) -> p o w", o=1)
                        .to_broadcast([128, GC, 128]), op=ALU.is_equal)
                    # transposed one-hot via PE row-broadcast of dfFlat + is_equal
                    OH2 = sbg.tile([128, GC * 128], BF16, tag="OH2")
                    for q in range(GC * 128 // 512):
                        pbc = pst.tile([128, 512], F32, space="PSUM", tag="pbc")
                        nc.tensor.matmul(
                            pbc[:], lhsT=ones1[:],
                            rhs=dfF_t[0:1, c0 * 128 + q * 512:c0 * 128 + (q + 1) * 512],
                            start=True, stop=True)
                        nc.vector.tensor_tensor(
                            out=OH2[:, q * 512:(q + 1) * 512], in0=pbc[:],
                            in1=iotaP_t[:].to_broadcast([128, 512]),
                            op=ALU.is_equal)
                    ADP = psd.tile([128, GC * HEADS], F32, space="PSUM", tag="adp")
                    for cc in range(GC):
                        cg = c0 + cc
                        nc.tensor.matmul(
                            ADP[:, cc * HEADS:(cc + 1) * HEADS],
                            lhsT=OH2[:, cc * 128:(cc + 1) * 128],
                            rhs=adS[:, (cg // 2) * HEADS:(cg // 2 + 1) * HEADS],
                            start=True, stop=True)
                    # ---- batched score math for this group (fp32) ----
                    ADE = sbg.tile([128, GC * HEADS], F32, tag="ADE")
                    nc.vector.tensor_copy(ADE[:], ADP[:])
                    ASR = sbg.tile([128, GC * HEADS], F32, tag="ASR")
                    nc.vector.tensor_copy(
                        ASR[:].rearrange("p (e w) -> p e w", w=HEADS),
                        NHv[:, c0:c0 + GC, 128:132])
                    EAm = sbg.tile([128, GC], F32, tag="EAm")
                    nc.vector.scalar_tensor_tensor(
                        out=EAm[:], in0=sm_t[:, c0:c0 + GC], scalar=meanr[:],
                        op0=ALU.mult, op1=ALU.add, in1=ea_t[:, c0:c0 + GC])
                    Q = sbg.tile([128, GC * HEADS], F32, tag="Q")
                    Qv = Q[:].rearrange("p (e w) -> p e w", w=HEADS)
                    nc.vector.tensor_tensor(out=Q[:], in0=ASR[:], in1=ADE[:],
                                            op=ALU.add)
                    T2 = sbg.tile([128, GC * HEADS], F32, tag="T2")
                    T2v = T2[:].rearrange("p (e w) -> p e w", w=HEADS)
                    ea3 = EAm[:].rearrange("p (e w) -> p e w", w=1) \
                        .to_broadcast([128, GC, HEADS])
                    kap3 = kap_t[:].rearrange("p (o w) -> p o w", o=1) \
                        .to_broadcast([128, GC, HEADS])
                    nc.vector.tensor_tensor(out=T2v, in0=ea3, in1=kap3, op=ALU.mult)
                    nc.vector.tensor_tensor(out=Q[:], in0=Q[:], in1=T2[:], op=ALU.add)
                    nc.vector.tensor_scalar_mul(T2[:], Q[:], LEAK)
                    nc.vector.tensor_tensor(out=Q[:], in0=Q[:], in1=T2[:], op=ALU.max)
                    # exp(q) = sigmoid(q)/sigmoid(-q); Exp ACT table not resident
                    SG1 = sbg.tile([128, GC * HEADS], F32, tag="SG1")
                    nc.scalar.activation(SG1[:], Q[:], AF.Sigmoid)
                    nc.scalar.activation(T2[:], Q[:], AF.Sigmoid, scale=-1.0)
                    nc.vector.reciprocal(T2[:], T2[:])
                    nc.vector.tensor_tensor(out=Q[:], in0=SG1[:], in1=T2[:],
                                            op=ALU.mult)
                    S4b = sbg.tile([128, GC * HEADS], BF16, tag="S4b")
                    nc.vector.tensor_copy(S4b[:], Q[:])
                    S4v = S4b[:].rearrange("p (e w) -> p e w", w=HEADS)
                    nc.vector.tensor_copy(NHv[:, c0:c0 + GC, 128:132], S4v)
                    nh4 = NHv[:, c0:c0 + GC, 0:128] \
                        .rearrange("p e (h c) -> p e h c", h=HEADS)
                    sc4 = S4v.rearrange("p e (h c) -> p e h c", c=1) \
                        .to_broadcast([128, GC, HEADS, C])
                    nc.vector.tensor_tensor(out=nh4, in0=nh4, in1=sc4, op=ALU.mult)
                    # ---- aggregation: 2 chunks per bucket, 2 buckets per PSUM
                    for b4 in range(GC // 4):
                        bk0 = gp * (GC // 2) + 2 * b4
                        pacc = pse.tile([128, 2 * W132], F32, space="PSUM", tag="pacc")
                        for bo in range(2):
                            bkt = bk0 + bo
                            sl_ = pacc[:, bo * W132:(bo + 1) * W132]
                            nc.tensor.matmul(sl_, lhsT=OHv[:, 2 * bkt, :],
                                             rhs=NHv[:, 2 * bkt, :], start=True,
                                             stop=False)
                            nc.tensor.matmul(sl_, lhsT=OHv[:, 2 * bkt + 1, :],
                                             rhs=NHv[:, 2 * bkt + 1, :], start=False,
                                             stop=True)
                        nc.vector.tensor_copy(
                            ACCT[:, bk0 * W132:(bk0 + 2) * W132], pacc[:])
                nc.sync.dma_start(acc_tbl[:], ACCT[:])

        if STAGE >= 3:
            # ---------- A6: ReduceScatter (bf16) ----------
            nc.gpsimd.collective_compute("ReduceScatter", ALU.add, replica_groups=RG,
                                         ins=[acc_tbl[:]], outs=[rs_out[:]])

            # ---------- A7: normalize + bias + ReLU at full width ----------
            with ExitStack() as ph:
                sbn = ph.enter_context(tc.tile_pool(name="sbn", bufs=1))
                RSS = sbn.tile([128, 10 * W132], BF16)
                nc.sync.dma_start(
                    RSS[:], rs_out[:].rearrange("r (m f) -> (r m) f", f=10 * W132))
                RSF = sbn.tile([128, 10 * W132], F32)
                nc.vector.tensor_copy(RSF[:], RSS[:])
                RSv = RSF[:].rearrange("p (j w) -> p j w", w=W132)
                DEN = sbn.tile([128, 10 * HEADS], F32)
                DENv = DEN[:].rearrange("p (j w) -> p j w", w=HEADS)
                nc.vector.tensor_scalar_add(DENv, RSv[:, :, 128:132], 1e-16)
                nc.vector.reciprocal(DEN[:], DEN[:])
                r4 = DENv.rearrange("p j (h c) -> p j h c", c=1) \
                    .to_broadcast([128, 10, HEADS, C])
                m4 = RSv[:, :, 0:128].rearrange("p j (h c) -> p j h c", h=HEADS)
                nc.vector.tensor_tensor(out=m4, in0=m4, in1=r4, op=ALU.mult)
                gbb = gb_t[:, 0:128].rearrange("p (o w) -> p o w", o=1) \
                    .to_broadcast([128, 10, 128])
                nc.vector.tensor_tensor(out=RSv[:, :, 0:128], in0=RSv[:, :, 0:128],
                                        in1=gbb, op=ALU.add)
                nc.vector.tensor_scalar_max(RSv[:, :, 0:128], RSv[:, :, 0:128], 0.0)
                nc.sync.dma_start(
                    norm_tbl[:].rearrange("r (m f) -> (r m) f", f=10 * W132), RSF[:])

            # ---------- A7b: transpose own slice into GEXT ----------
            with ExitStack() as ph:
                sbt = ph.enter_context(tc.tile_pool(name="sbt", bufs=1))
                psn = ph.enter_context(tc.tile_pool(name="psn", bufs=4, space="PSUM"))
                TRS = sbt.tile([16, DBLK * W132], F32)
                nc.sync.dma_start(TRS[:], norm_tbl[:])
                TRv = TRS[:].rearrange("p (j w) -> p j w", w=W132)
                for j0 in range(0, DBLK, 4):
                    ptr = psn.tile([128, 64], F32, space="PSUM", tag="ptr")
                    for j in range(j0, j0 + 4):
                        nc.tensor.transpose(out=ptr[:, (j - j0) * 16:(j - j0 + 1) * 16],
                                            in_=TRv[:, j, 0:128],
                                            identity=ident[0:16, 0:16])
                    nc.vector.tensor_copy(
                        GEXT[:, HALO + j0 * 16:HALO + (j0 + 4) * 16], ptr[:])
                nc.sync.dma_start(tail_blob[:], GEXT[:, SL:SL + HALO])

        if STAGE >= 4:
            # ---------- A8: tail AllGather + halo fetch ----------
            nc.gpsimd.collective_compute("AllGather", ALU.bypass, replica_groups=RG,
                                         ins=[tail_blob[:]], outs=[tails_full[:]])
            nc.gpsimd.indirect_dma_start(
                out=GEXT[:, 0:HALO], out_offset=None, in_=tails_full[:],
                in_offset=bass.IndirectOffsetOnAxis(ap=haloI_t[:], axis=0))
        if debug:
            dbgG = persist.tile([128, NCOL], F32)
            nc.vector.tensor_copy(dbgG[:], GEXT[:])
            nc.sync.dma_start(dbg_gat[:], dbgG[:])

        if STAGE >= 5:
            # ---------- A9: gx = W_ih @ gat + b; halo fix ----------
            with ExitStack() as ph:
                psg = ph.enter_context(tc.tile_pool(name="psg", bufs=4, space="PSUM"))
                for c, (lo, hi) in enumerate([(0, 512), (512, 1024), (1024, NCOL)]):
                    pg = psg.tile([128, hi - lo], F32, space="PSUM", tag="pg")
                    nc.tensor.matmul(pg[:], lhsT=wih_t[:], rhs=GEXT[:, lo:hi],
                                     start=True, stop=True)
                    nc.vector.tensor_scalar_add(gx[:, lo:hi], pg[:], br_t[:])
                nc.vector.scalar_tensor_tensor(
                    out=gx[:, 0:HALO], in0=gx[:, 0:HALO], scalar=haloM_t[:],
                    op0=ALU.mult, op1=ALU.add, in1=haloF_t[:])
        if debug:
            nc.sync.dma_start(dbg_gx[:], gx[:])

        if STAGE >= 6:
            # ---------- A10: LSTM fixed point, single 1408-col chain ------
            with ExitStack() as ph:
                sbl = ph.enter_context(tc.tile_pool(name="sbl", bufs=2))
                psl = ph.enter_context(tc.tile_pool(name="psl", bufs=4, space="PSUM"))
                for it in range(ITERS):
                    if it == 0:
                        Gp = gx[:]
                    else:
                        Gs = sbl.tile([128, NCOL], F32, tag="Gs")
                        for lo, hi in [(0, 512), (512, 1024), (1024, NCOL)]:
                            pG = psl.tile([128, hi - lo], F32, space="PSUM", tag="pG")
                            nc.tensor.matmul(pG[:], lhsT=whh_t[:], rhs=H[:, lo:hi],
                                             start=True, stop=True)
                            nc.vector.tensor_tensor(out=Gs[:, lo:hi], in0=pG[:],
                                                    in1=gx[:, lo:hi], op=ALU.add)
                        Gp = Gs[:]
                    S_ = sbl.tile([96, NCOL], F32, tag="S")
                    nc.scalar.activation(S_[:], Gp[0:96, :], AF.Sigmoid)
                    Tg = sbl.tile([64, NCOL], F32, tag="Tg")
                    nc.scalar.activation(Tg[32:64, :], Gp[96:128, :], AF.Tanh)
                    Zt = sbl.tile([HID, NCOL], F32, tag="Zt")
                    nc.vector.tensor_tensor(out=Zt[:], in0=S_[32:64, :],
                                            in1=Tg[32:64, :], op=ALU.mult)
                    Ct = sbl.tile([HID, NCOL], F32, tag="Ct")
                    nc.vector.tensor_tensor_scan(
                        out=Ct[:], data0=S_[0:32, :], data1=Zt[:], initial=0.0,
                        op0=ALU.mult, op1=ALU.add)
                    TC = sbl.tile([96, NCOL], F32, tag="TC")
                    nc.scalar.activation(TC[64:96, :], Ct[:], AF.Tanh)
                    nc.vector.tensor_tensor(out=H[:, 1:NCOL + 1], in0=S_[64:96, :],
                                            in1=TC[64:96, :], op=ALU.mult)
        if debug:
            dbgH = persist.tile([HID, NCOL], F32)
            nc.vector.tensor_copy(dbgH[:], H[:, 1:NCOL + 1])
            nc.sync.dma_start(dbg_h[:], dbgH[:])

        if STAGE >= 7:
            # ---------- FC on own 1280 columns ----------
            with ExitStack() as ph:
                sbf = ph.enter_context(tc.tile_pool(name="sbf", bufs=1))
                psf = ph.enter_context(tc.tile_pool(name="psf", bufs=4, space="PSUM"))
                OF = sbf.tile([1, SL], F32)
                for c, (lo, hi) in enumerate([(0, 512), (512, 1024), (1024, SL)]):
                    pf = psf.tile([1, hi - lo], F32, space="PSUM", tag="pf")
                    nc.tensor.matmul(pf[:], lhsT=wfc_t[:],
                                     rhs=H[:, HALO + 1 + lo:HALO + 1 + hi],
                                     start=True, stop=True)
                    nc.vector.tensor_scalar_add(OF[:, lo:hi], pf[:], bfc_t[:])
                nc.sync.dma_start(out[:], OF[:])

    nc.compile()
    return nc


def run(inputs, trace=False, debug=False):
    key = ("dbg" if debug else "rel")
    if key not in _CACHE:
        _CACHE[key] = _build_nc(debug=debug)
    nc = _CACHE[key]
    in_maps = _prep_host(inputs)
    res = run_bass_kernel_spmd(nc, in_maps, list(range(NCORES)), trace=trace)
    return res


def kernel(**inputs) -> np.ndarray:
    res = run(inputs)
    o = np.concatenate([np.asarray(res.results[k]["out"][0], np.float32)
                        for k in range(NCORES)])
    return np.ascontiguousarray(o[:N].reshape(N, 1))
